# revision 1
# baseline (speedup 1.0000x reference)
"""AttnDecoderRNN single-step on 8 trn2 NeuronCores.

Strategy (tensor parallel over vocab, per sharding hint):
- Embedding lookup on host (only the one needed row of the 206MB table is
  ever read; shipping it to the device would be pure waste).
- Attention + combine replicated on every core (small weights).
- LSTM cell sharded over the hidden dim: core r computes gate slices
  i/f/g/o[r*128:(r+1)*128] -> its h/c slice.
- AllGather of the 128-elem h slices -> full h on every core.
- Output projection sharded over vocab: core r computes logits for its
  ~6.6k vocab columns. log_softmax normalization is finished on host
  (combining 8 shard stats; O(vocab) host work).
All matvecs run the vector as the PE stationary operand and stream the
weight matrix as the moving operand; weights are host-packed into the
exact SBUF layout so every weight DMA is a single contiguous transfer.
"""

import os
import numpy as np
import ml_dtypes

import concourse.bass as bass
import concourse.bacc as bacc
import concourse.mybir as mybir
import concourse.tile as tile
from concourse import bass_utils

NCORES = 8
H = 1024
SEQ = 512
VOCAB = 50257
VS = 6656               # padded per-core vocab shard; 8*VS = 53248
NVT = VS // 512         # 13 weight panels per core
KD = os.environ.get("KDTYPE", "bf16")
NPW = ml_dtypes.bfloat16 if KD == "bf16" else np.float32

_cache = {}


def _pack_kxm(vec):
    """[C*128] vector -> [128, C] chunk-per-column layout (fp PE stationary)."""
    c = vec.shape[0] // 128
    return np.ascontiguousarray(vec.reshape(c, 128).T).astype(NPW)


def _pack_w(wT, n_block):
    """[K, N] weight (K = contraction, multiple of 128) -> [128, (K/128)*N]
    where column block k holds rows k*128:(k+1)*128. n_block unused pad hook."""
    k = wT.shape[0] // 128
    n = wT.shape[1]
    return np.ascontiguousarray(
        wT.reshape(k, 128, n).transpose(1, 0, 2).reshape(128, k * n)
    ).astype(NPW)


def _build():
    wdt = mybir.dt.bfloat16 if KD == "bf16" else mybir.dt.float32
    f32 = mybir.dt.float32
    AF = mybir.ActivationFunctionType
    OP = mybir.AluOpType

    nc = bacc.Bacc("TRN2", target_bir_lowering=False, debug=False,
                   num_devices=NCORES)

    def din(name, shape, dt=None):
        return nc.dram_tensor(name, shape, dt or wdt, kind="ExternalInput").ap()

    def dout(name, shape):
        return nc.dram_tensor(name, shape, f32, kind="ExternalOutput").ap()

    d_ain = din("ain", [128, 16])                 # concat(embedded, h0) kxm
    d_attnw = din("attn_w", [128, 16 * 512])
    d_attnb = din("attn_b", [1, 512], f32)
    d_enc = din("enc", [128, 4 * 1024])
    d_combw = din("comb_w", [128, 16 * 1024])
    d_combb = din("comb_b", [1, 1024], f32)
    d_lstmw = din("lstm_w", [128, 16 * 512])
    d_lstmb = din("lstm_b", [1, 512], f32)
    d_c0 = din("c0s", [1, 128], f32)
    d_wout = din("wout", [NVT, 128, 8 * 512])
    d_outb = din("out_b", [1, VS], f32)

    d_logits = dout("logits_o", [1, VS])
    d_ho = dout("h_o", [1, 128])
    d_co = dout("c_o", [1, 128])
    d_awo = dout("attnw_o", [1, 512])

    with tile.TileContext(nc) as tc:
        with (
            tc.tile_pool(name="const", bufs=1) as cpool,
            tc.tile_pool(name="wpool", bufs=6) as wpool,
            tc.tile_pool(name="act", bufs=1) as apool,
            tc.tile_pool(name="ps", bufs=1, space="PSUM") as pspool,
            tc.tile_pool(name="pso", bufs=2, space="PSUM") as psopool,
            tc.tile_pool(name="dram", bufs=1, space="DRAM") as dpool,
        ):
            # ---------- static loads (issued up front; DMA streams during compute)
            ain = cpool.tile([128, 16], wdt)
            nc.sync.dma_start(ain[:], d_ain[:])
            aw = cpool.tile([128, 16 * 512], wdt)
            nc.sync.dma_start(aw[:], d_attnw[:])
            enc = cpool.tile([128, 4 * 1024], wdt)
            nc.sync.dma_start(enc[:], d_enc[:])
            cw = cpool.tile([128, 16 * 1024], wdt)
            nc.sync.dma_start(cw[:], d_combw[:])
            lw = cpool.tile([128, 16 * 512], wdt)
            nc.sync.dma_start(lw[:], d_lstmw[:])
            ab = cpool.tile([1, 512], f32)
            nc.sync.dma_start(ab[:], d_attnb[:])
            cb = cpool.tile([1, 1024], f32)
            nc.sync.dma_start(cb[:], d_combb[:])
            lb = cpool.tile([1, 512], f32)
            nc.sync.dma_start(lb[:], d_lstmb[:])
            c0s = cpool.tile([1, 128], f32)
            nc.sync.dma_start(c0s[:], d_c0[:])
            ob = cpool.tile([1, VS], f32)
            nc.sync.dma_start(ob[:], d_outb[:])

            def to_kxm(src_sb_f32, n, cast_name):
                """[1, n*128] f32 sbuf -> [128, n] wdt sbuf via DRAM bounce."""
                cst = apool.tile([1, n * 128], wdt, name=cast_name + "_c")
                nc.vector.tensor_copy(cst[:], src_sb_f32)
                bnc = dpool.tile([n, 128], wdt, name=cast_name + "_b")
                nc.sync.dma_start(bnc[:], cst[:])
                kxm = apool.tile([128, n], wdt, name=cast_name + "_k")
                for c in range(n):
                    nc.sync.dma_start(kxm[:, c:c + 1], bnc[c, :])
                return kxm

            # ---------- stage 1: attention scores + softmax  [1,512]
            ps_s = pspool.tile([1, 512], f32)
            for k in range(16):
                nc.tensor.matmul(ps_s[:], ain[:, k:k + 1],
                                 aw[:, k * 512:(k + 1) * 512],
                                 start=(k == 0), stop=(k == 15))
            scores = apool.tile([1, 512], f32)
            nc.vector.tensor_add(scores[:], ps_s[:], ab[:])
            mx = apool.tile([1, 1], f32)
            nc.vector.reduce_max(mx[:], scores[:], axis=mybir.AxisListType.X)
            nmx = apool.tile([1, 1], f32)
            nc.vector.tensor_scalar_mul(nmx[:], mx[:], -1.0)
            expv = apool.tile([1, 512], f32)
            ssum = apool.tile([1, 1], f32)
            nc.scalar.activation(expv[:], scores[:], AF.Exp, bias=nmx[:],
                                 scale=1.0, accum_out=ssum[:])
            rs = apool.tile([1, 1], f32)
            nc.vector.reciprocal(rs[:], ssum[:])
            awf = apool.tile([1, 512], f32)
            nc.vector.tensor_scalar_mul(awf[:], expv[:], rs[:])
            nc.sync.dma_start(d_awo[:], awf[:])
            aw_kxm = to_kxm(awf[:], 4, "awr")

            # ---------- stage 2: attn_applied [1,1024]
            ps_a = pspool.tile([1, 1024], f32)
            for n in range(2):
                for k in range(4):
                    nc.tensor.matmul(
                        ps_a[:, n * 512:(n + 1) * 512], aw_kxm[:, k:k + 1],
                        enc[:, k * 1024 + n * 512: k * 1024 + (n + 1) * 512],
                        start=(k == 0), stop=(k == 3))
            aa_kxm = to_kxm(ps_a[:], 8, "aar")

            # ---------- stage 3: combine + relu -> x [1,1024]
            ps_x = pspool.tile([1, 1024], f32)
            for n in range(2):
                for k in range(16):
                    lhs = ain[:, k:k + 1] if k < 8 else aa_kxm[:, k - 8:k - 7]
                    nc.tensor.matmul(
                        ps_x[:, n * 512:(n + 1) * 512], lhs,
                        cw[:, k * 1024 + n * 512: k * 1024 + (n + 1) * 512],
                        start=(k == 0), stop=(k == 15))
            xb = apool.tile([1, 1024], f32)
            nc.vector.tensor_add(xb[:], ps_x[:], cb[:])
            xr = apool.tile([1, 1024], f32)
            nc.vector.tensor_scalar_max(xr[:], xb[:], 0.0)
            x_kxm = to_kxm(xr[:], 8, "xr")

            # ---------- stage 4: LSTM gate slices [1,512] = [i|f|g|o]x128
            ps_g = pspool.tile([1, 512], f32)
            for k in range(16):
                lhs = x_kxm[:, k:k + 1] if k < 8 else ain[:, k:k + 1]
                nc.tensor.matmul(ps_g[:], lhs, lw[:, k * 512:(k + 1) * 512],
                                 start=(k == 0), stop=(k == 15))
            gt = apool.tile([1, 512], f32)
            nc.vector.tensor_add(gt[:], ps_g[:], lb[:])
            acts = apool.tile([1, 512], f32)
            nc.scalar.activation(acts[:, 0:256], gt[:, 0:256], AF.Sigmoid)
            nc.scalar.activation(acts[:, 256:384], gt[:, 256:384], AF.Tanh)
            nc.scalar.activation(acts[:, 384:512], gt[:, 384:512], AF.Sigmoid)
            fc = apool.tile([1, 128], f32)
            nc.vector.tensor_mul(fc[:], acts[:, 128:256], c0s[:])
            ig = apool.tile([1, 128], f32)
            nc.vector.tensor_mul(ig[:], acts[:, 0:128], acts[:, 256:384])
            cn = apool.tile([1, 128], f32)
            nc.vector.tensor_add(cn[:], fc[:], ig[:])
            tch = apool.tile([1, 128], f32)
            nc.scalar.activation(tch[:], cn[:], AF.Tanh)
            hn = apool.tile([1, 128], f32)
            nc.vector.tensor_mul(hn[:], acts[:, 384:512], tch[:])
            nc.sync.dma_start(d_co[:], cn[:])
            nc.sync.dma_start(d_ho[:], hn[:])

            # ---------- AllGather h slices -> full h (kxm [128, 8])
            hc = apool.tile([1, 128], wdt)
            nc.vector.tensor_copy(hc[:], hn[:])
            ag_in = dpool.tile([1, 128], wdt)
            ag_out = dpool.tile([8, 128], wdt, addr_space="Shared")
            nc.sync.dma_start(ag_in[:], hc[:])
            nc.gpsimd.collective_compute(
                "AllGather", OP.bypass,
                replica_groups=[list(range(NCORES))],
                ins=[ag_in[:]], outs=[ag_out[:]])
            h_kxm = apool.tile([128, 8], wdt)
            for c in range(8):
                nc.sync.dma_start(h_kxm[:, c:c + 1], ag_out[c, :])

            # ---------- stage 5: vocab-sharded output projection
            logits = apool.tile([1, VS], f32)
            for v in range(NVT):
                wp = wpool.tile([128, 8 * 512], wdt, name="wp")
                nc.sync.dma_start(wp[:], d_wout[v])
                ps_o = psopool.tile([1, 512], f32, name="ps_o")
                for k in range(8):
                    nc.tensor.matmul(ps_o[:], h_kxm[:, k:k + 1],
                                     wp[:, k * 512:(k + 1) * 512],
                                     start=(k == 0), stop=(k == 7))
                nc.vector.tensor_add(logits[:, v * 512:(v + 1) * 512],
                                     ps_o[:], ob[:, v * 512:(v + 1) * 512])
            nc.sync.dma_start(d_logits[:], logits[:])

    nc.compile()
    return nc


def _prep_inputs(input_idx, h0, c0, encoder_outputs, emb, attn_W, attn_b,
                 comb_W, comb_b, w_ih, w_hh, b_ih, b_hh, out_W, out_b):
    f = np.float32
    idx = int(np.asarray(input_idx).reshape(-1)[0])
    embedded = np.asarray(emb, f)[idx]                    # [H]
    h0v = np.asarray(h0, f).reshape(H)
    c0v = np.asarray(c0, f).reshape(H)

    ain = _pack_kxm(np.concatenate([embedded, h0v]))      # [128, 16]
    attn_w = _pack_w(np.asarray(attn_W, f).T, 1)          # [128, 16*512]
    enc = _pack_w(np.asarray(encoder_outputs, f), 1)      # [128, 4*1024]
    comb_w = _pack_w(np.asarray(comb_W, f).T, 1)          # [128, 16*1024]
    attn_bv = np.asarray(attn_b, f).reshape(1, 512)
    comb_bv = np.asarray(comb_b, f).reshape(1, 1024)

    w_cat = np.concatenate([np.asarray(w_ih, f).T,
                            np.asarray(w_hh, f).T], axis=0)   # [2048, 4096]
    b_cat = (np.asarray(b_ih, f) + np.asarray(b_hh, f))       # [4096]

    out_WT = np.asarray(out_W, f).T                       # [1024, VOCAB]
    out_WT_pad = np.zeros((H, NCORES * VS), f)
    out_WT_pad[:, :VOCAB] = out_WT
    out_b_pad = np.zeros(NCORES * VS, f)
    out_b_pad[:VOCAB] = np.asarray(out_b, f)

    in_maps = []
    for r in range(NCORES):
        cols = np.concatenate(
            [np.arange(g * H + r * 128, g * H + (r + 1) * 128)
             for g in range(4)])
        lstm_w = _pack_w(np.ascontiguousarray(w_cat[:, cols]), 1)  # [128,16*512]
        lstm_b = b_cat[cols].reshape(1, 512)
        wv = out_WT_pad[:, r * VS:(r + 1) * VS]           # [1024, VS]
        wout = np.ascontiguousarray(
            wv.reshape(8, 128, NVT, 512).transpose(2, 1, 0, 3)
        ).reshape(NVT, 128, 8 * 512).astype(NPW)
        in_maps.append({
            "ain": ain, "attn_w": attn_w, "attn_b": attn_bv,
            "enc": enc, "comb_w": comb_w, "comb_b": comb_bv,
            "lstm_w": lstm_w, "lstm_b": lstm_b,
            "c0s": c0v[r * 128:(r + 1) * 128].reshape(1, 128),
            "wout": wout,
            "out_b": out_b_pad[r * VS:(r + 1) * VS].reshape(1, VS),
        })
    return in_maps


def run_on_device(in_maps, trace=False):
    if "nc" not in _cache:
        _cache["nc"] = _build()
    nc = _cache["nc"]
    return bass_utils.run_bass_kernel_spmd(
        nc, in_maps, core_ids=list(range(NCORES)), trace=trace)


def kernel(**inputs):
    in_maps = _prep_inputs(**inputs)
    res = run_on_device(in_maps).results

    logits = np.concatenate(
        [res[r]["logits_o"].reshape(VS) for r in range(NCORES)])[:VOCAB]
    m = float(logits.max())
    lse = m + float(np.log(np.exp(logits - m, dtype=np.float64).sum()))
    out = (logits - np.float32(lse)).astype(np.float32).reshape(1, VOCAB)

    h = np.concatenate([res[r]["h_o"].reshape(128) for r in range(NCORES)])
    c = np.concatenate([res[r]["c_o"].reshape(128) for r in range(NCORES)])
    attnw = res[0]["attnw_o"].reshape(1, SEQ).astype(np.float32)
    return (out,
            h.astype(np.float32).reshape(1, 1, H),
            c.astype(np.float32).reshape(1, 1, H),
            attnw)


# revision 13
# speedup vs baseline: 1.1081x; 1.1081x over previous
"""AttnDecoderRNN single-step on 8 trn2 NeuronCores.

Strategy (tensor parallel over vocab, per sharding hint):
- Embedding lookup on host (only the one needed row of the 206MB table is
  ever read; shipping the table to the device would be pure waste).
- Attention + combine replicated on every core (small weights).
- LSTM cell sharded over the hidden dim: core r computes gate slices
  i/f/g/o[r*128:(r+1)*128] -> its h/c slice.
- AllGather of the 128-elem h slices -> full h on every core.
- Output projection sharded over vocab: core r computes logits for its
  6656 (padded) vocab columns. log_softmax normalization is finished on
  host (combines 8 shard stats; O(vocab) host work).

All matvecs run the activation vector as the PE stationary operand and
stream the weight matrix as the moving operand. Weights are host-packed
into the exact SBUF layout and streamed in [128, 4096] chunks through one
tile pool, so weight DMA pipelines with PE and the out-projection panels
prefetch during phase 1. Vector transposes ([1,n*128] -> [128,n]) are done
on the PE via K=1 matmuls against a ones scalar (no DRAM bounce).
"""

import os
import numpy as np
import ml_dtypes

import concourse.bass as bass
import concourse.bacc as bacc
import concourse.mybir as mybir
import concourse.tile as tile
from concourse import bass_utils

NCORES = 8
H = 1024
SEQ = 512
VOCAB = 50257
VS = 6656               # padded per-core vocab shard; 8*VS = 53248
NVT = VS // 512         # 13 weight panels of [128, 8*512] per core
CHUNK = 4096            # streaming chunk free-dim (8KB/partition bf16)
KD = os.environ.get("KDTYPE", "bf16")
NPW = ml_dtypes.bfloat16 if KD == "bf16" else np.float32
WBUFS = int(os.environ.get("KWBUFS", "21"))

_cache = {}


def _pack_kxm(vec):
    """[C*128] vector -> [128, C] chunk-per-column layout (PE stationary)."""
    c = vec.shape[0] // 128
    return np.ascontiguousarray(vec.reshape(c, 128).T).astype(NPW)


def _pack_w(wT):
    """[K, N] weight (K = contraction, mult of 128) -> [128, (K/128)*N];
    column block k holds rows k*128:(k+1)*128."""
    k = wT.shape[0] // 128
    n = wT.shape[1]
    return np.ascontiguousarray(
        wT.reshape(k, 128, n).transpose(1, 0, 2).reshape(128, k * n)
    ).astype(NPW)


def _build(ablate=()):
    wdt = mybir.dt.bfloat16 if KD == "bf16" else mybir.dt.float32
    f32 = mybir.dt.float32
    AF = mybir.ActivationFunctionType
    OP = mybir.AluOpType

    nc = bacc.Bacc("TRN2", target_bir_lowering=False, debug=False,
                   num_devices=NCORES)

    def din(name, shape, dt=None):
        return nc.dram_tensor(name, shape, dt or wdt, kind="ExternalInput").ap()

    def dout(name, shape):
        return nc.dram_tensor(name, shape, f32, kind="ExternalOutput").ap()

    # cst_all: cols 0:16 = ain kxm, col 16 = ones, cols 17:25 rows 0:8 = eye8
    d_cst = din("cst_all", [128, 32])
    # bias_all: 0:512 attn_b | 512:1536 comb_b | 1536:2048 lstm_b | 2048:2176 c0
    d_bias = din("bias_all", [1, 2176], f32)
    d_attnw = din("attn_w", [128, 16 * 512])      # 2 chunks
    d_enc = din("enc", [128, 4 * 1024])           # 1 chunk
    d_combw = din("comb_w", [128, 16 * 1024])     # 4 chunks
    d_lstmw = din("lstm_w", [128, 16 * 512])      # 2 chunks
    d_wout = din("wout", [NVT, 128, 8 * 512])     # 13 panels

    d_logits = dout("logits_o", [1, VS])
    d_ho = dout("h_o", [1, 128])
    d_co = dout("c_o", [1, 128])
    d_awo = dout("attnw_o", [1, 512])

    with tile.TileContext(nc) as tc:
        with (
            tc.tile_pool(name="const", bufs=1) as cpool,
            tc.tile_pool(name="wpool", bufs=WBUFS) as wpool,
            tc.tile_pool(name="act", bufs=1) as apool,
            tc.tile_pool(name="ps", bufs=2, space="PSUM") as pspool,
            tc.tile_pool(name="pst", bufs=2, space="PSUM") as pstpool,
            tc.tile_pool(name="pso", bufs=2, space="PSUM") as psopool,
            tc.tile_pool(name="dram", bufs=1, space="DRAM") as dpool,
        ):
            # ---- two merged constant loads (unblock phase-1 chain fast)
            cst = cpool.tile([128, 32], wdt)
            nc.scalar.dma_start(cst[:], d_cst[:])
            bias = cpool.tile([1, 2176], f32)
            nc.scalar.dma_start(bias[:], d_bias[:])
            ain = cst[:, 0:16]
            one = cst[0:1, 16:17]
            eye8 = cst[0:8, 17:25]
            ab = bias[:, 0:512]
            cb = bias[:, 512:1536]
            lb = bias[:, 1536:2048]
            c0s = bias[:, 2048:2176]

            def wchunk(src_ap, name):
                t = wpool.tile([128, CHUNK], wdt, name=name, tag="ws")
                nc.sync.dma_start(t[:], src_ap)
                return t

            # ---- streamed weight chunks, program order = DMA priority
            aw0 = wchunk(d_attnw[:, 0:CHUNK], "aw0")
            aw1 = wchunk(d_attnw[:, CHUNK:2 * CHUNK], "aw1")
            enc = wchunk(d_enc[:], "enc")
            cwc = [wchunk(d_combw[:, c * CHUNK:(c + 1) * CHUNK], f"cw{c}")
                   for c in range(4)]
            lw0 = wchunk(d_lstmw[:, 0:CHUNK], "lw0")
            lw1 = wchunk(d_lstmw[:, CHUNK:2 * CHUNK], "lw1")
            NPRE = int(os.environ.get("KNPRE", "5"))
            wps = [wchunk(d_wout[v], f"wp{v}") for v in range(NPRE)]

            def vec_to_kxm(src_f32_ap, n, name):
                """[1, n*128] f32 -> [128, n] wdt via K=1 PE matmuls."""
                vc = apool.tile([1, n * 128], wdt, name=name + "_c")
                nc.vector.tensor_copy(vc[:], src_f32_ap)
                pst = pstpool.tile([128, 8], f32, name=name + "_p", tag="pst")
                for c in range(n):
                    nc.tensor.matmul(pst[:, c:c + 1],
                                     vc[:, c * 128:(c + 1) * 128], one,
                                     start=True, stop=True)
                kxm = apool.tile([128, n], wdt, name=name + "_k")
                nc.vector.tensor_copy(kxm[:], pst[:, 0:n])
                return kxm

            # ---------- stage 1: attention scores + softmax [1,512]
            ps_s = pspool.tile([1, 1024], f32, name="ps_s", tag="ps")
            for k in range(16):
                src = (aw0 if k < 8 else aw1)
                kk = k % 8
                nc.tensor.matmul(ps_s[:, 0:512], ain[:, k:k + 1],
                                 src[:, kk * 512:(kk + 1) * 512],
                                 start=(k == 0), stop=(k == 15))
            scores = apool.tile([1, 512], f32)
            nc.vector.tensor_add(scores[:], ps_s[:, 0:512], ab)
            mx = apool.tile([1, 1], f32)
            nc.vector.reduce_max(mx[:], scores[:], axis=mybir.AxisListType.X)
            nmx = apool.tile([1, 1], f32)
            nc.vector.tensor_scalar_mul(nmx[:], mx[:], -1.0)
            expv = apool.tile([1, 512], f32)
            ssum = apool.tile([1, 1], f32)
            nc.scalar.activation(expv[:], scores[:], AF.Exp, bias=nmx[:],
                                 scale=1.0, accum_out=ssum[:])
            rs = apool.tile([1, 1], f32)
            nc.vector.reciprocal(rs[:], ssum[:])
            awf = apool.tile([1, 512], f32)
            nc.vector.tensor_scalar_mul(awf[:], expv[:], rs[:])
            aw_kxm = vec_to_kxm(awf[:], 4, "awr")

            # ---------- stage 2: attn_applied [1,1024]
            ps_a = pspool.tile([1, 1024], f32, name="ps_a", tag="ps")
            for n in range(2):
                for k in range(4):
                    nc.tensor.matmul(
                        ps_a[:, n * 512:(n + 1) * 512], aw_kxm[:, k:k + 1],
                        enc[:, k * 1024 + n * 512: k * 1024 + (n + 1) * 512],
                        start=(k == 0), stop=(k == 3))
            aa_kxm = vec_to_kxm(ps_a[:], 8, "aar")

            # ---------- stage 3: combine + relu -> x [1,1024]
            ps_x = pspool.tile([1, 1024], f32, name="ps_x", tag="ps")
            for n in range(2):
                for k in range(16):
                    lhs = ain[:, k:k + 1] if k < 8 else aa_kxm[:, k - 8:k - 7]
                    c, kk = divmod(k * 1024 + n * 512, CHUNK)
                    nc.tensor.matmul(
                        ps_x[:, n * 512:(n + 1) * 512], lhs,
                        cwc[c][:, kk:kk + 512],
                        start=(k == 0), stop=(k == 15))
            xb = apool.tile([1, 1024], f32)
            nc.vector.tensor_add(xb[:], ps_x[:], cb)
            xr = apool.tile([1, 1024], f32)
            nc.vector.tensor_scalar_max(xr[:], xb[:], 0.0)
            x_kxm = vec_to_kxm(xr[:], 8, "xr")

            # ---------- stage 4: LSTM gate slices [1,512] = [i|f|g|o]x128
            ps_g = pspool.tile([1, 1024], f32, name="ps_g", tag="ps")
            for k in range(16):
                lhs = x_kxm[:, k:k + 1] if k < 8 else ain[:, k:k + 1]
                src = lw0 if k < 8 else lw1
                kk = k % 8
                nc.tensor.matmul(ps_g[:, 0:512], lhs,
                                 src[:, kk * 512:(kk + 1) * 512],
                                 start=(k == 0), stop=(k == 15))
            gt = apool.tile([1, 512], f32)
            nc.vector.tensor_add(gt[:], ps_g[:, 0:512], lb)
            acts = apool.tile([1, 512], f32)
            nc.scalar.activation(acts[:, 0:256], gt[:, 0:256], AF.Sigmoid)
            nc.scalar.activation(acts[:, 256:384], gt[:, 256:384], AF.Tanh)
            nc.scalar.activation(acts[:, 384:512], gt[:, 384:512], AF.Sigmoid)
            fc = apool.tile([1, 128], f32)
            nc.vector.tensor_mul(fc[:], acts[:, 128:256], c0s)
            ig = apool.tile([1, 128], f32)
            nc.vector.tensor_mul(ig[:], acts[:, 0:128], acts[:, 256:384])
            cn = apool.tile([1, 128], f32)
            nc.vector.tensor_add(cn[:], fc[:], ig[:])
            tch = apool.tile([1, 128], f32)
            nc.scalar.activation(tch[:], cn[:], AF.Tanh)
            hn = apool.tile([1, 128], f32)
            hn_inst = nc.vector.tensor_mul(hn[:], acts[:, 384:512], tch[:])
            for v in range(NPRE, NVT):
                t = wpool.tile([128, CHUNK], wdt, name=f"wp{v}", tag="ws")
                dls = nc.sync.dma_start(t[:], d_wout[v])
                tile.add_dep_helper(getattr(dls, "ins", dls),
                                    getattr(hn_inst, "ins", hn_inst), sync=True)
                wps.append(t)

            # ---------- AllGather h slices -> full h, transpose to [128, 8]
            hc = apool.tile([1, 128], wdt)
            nc.vector.tensor_copy(hc[:], hn[:])
            ag_in = dpool.tile([1, 128], wdt)
            ag_out = dpool.tile([8, 128], wdt, addr_space="Shared")
            nc.gpsimd.dma_start(ag_in[:], hc[:])
            if "noag" not in ablate:
                nc.gpsimd.collective_compute(
                    "AllGather", OP.bypass,
                    replica_groups=[list(range(NCORES))],
                    ins=[ag_in[:]], outs=[ag_out[:]])
            hsb = apool.tile([8, 128], wdt)
            nc.gpsimd.dma_start(hsb[:], ag_out[:])
            ps_h = pstpool.tile([128, 8], f32, name="ps_h", tag="pst")
            nc.tensor.matmul(ps_h[:], hsb[:], eye8, start=True, stop=True)
            h_kxm = apool.tile([128, 8], wdt)
            nc.vector.tensor_copy(h_kxm[:], ps_h[:])

            nc.gpsimd.dma_start(d_awo[:], awf[:])
            nc.gpsimd.dma_start(d_co[:], cn[:])
            nc.gpsimd.dma_start(d_ho[:], hn[:])

            # ---------- stage 5: vocab-sharded output projection
            # (out_b is added on host with the shard-combining log_softmax)
            for v in range(NVT):
                ps_o = psopool.tile([1, 512], f32, name="ps_o", tag="pso")
                for k in range(8):
                    nc.tensor.matmul(ps_o[:], h_kxm[:, k:k + 1],
                                     wps[v][:, k * 512:(k + 1) * 512],
                                     start=(k == 0), stop=(k == 7))
                lst = apool.tile([1, 512], f32, name="lst", tag="lst", bufs=2)
                nc.vector.tensor_copy(lst[:], ps_o[:])
                nc.scalar.dma_start(d_logits[:, v * 512:(v + 1) * 512], lst[:])

    nc.compile()
    return nc


def _prep_inputs(input_idx, h0, c0, encoder_outputs, emb, attn_W, attn_b,
                 comb_W, comb_b, w_ih, w_hh, b_ih, b_hh, out_W, out_b):
    f = np.float32
    idx = int(np.asarray(input_idx).reshape(-1)[0])
    embedded = np.asarray(emb, f)[idx]                    # [H]
    h0v = np.asarray(h0, f).reshape(H)
    c0v = np.asarray(c0, f).reshape(H)

    ain = _pack_kxm(np.concatenate([embedded, h0v]))      # [128, 16]
    attn_w = _pack_w(np.asarray(attn_W, f).T)             # [128, 16*512]
    enc = _pack_w(np.asarray(encoder_outputs, f))         # [128, 4*1024]
    comb_w = _pack_w(np.asarray(comb_W, f).T)             # [128, 16*1024]
    attn_bv = np.asarray(attn_b, f).reshape(1, 512)
    comb_bv = np.asarray(comb_b, f).reshape(1, 1024)

    w_cat = np.concatenate([np.asarray(w_ih, f).T,
                            np.asarray(w_hh, f).T], axis=0)   # [2048, 4096]
    b_cat = (np.asarray(b_ih, f) + np.asarray(b_hh, f))       # [4096]

    out_WT = np.asarray(out_W, f).T                       # [1024, VOCAB]
    out_WT_pad = np.zeros((H, NCORES * VS), f)
    out_WT_pad[:, :VOCAB] = out_WT
    out_b_pad = np.zeros(NCORES * VS, f)
    out_b_pad[:VOCAB] = np.asarray(out_b, f)

    cst_all = np.zeros((128, 32), f)
    cst_all[:, 0:16] = ain.astype(f)
    cst_all[0, 16] = 1.0
    cst_all[0:8, 17:25] = np.eye(8, dtype=f)
    cst_all = cst_all.astype(NPW)

    in_maps = []
    for r in range(NCORES):
        cols = np.concatenate(
            [np.arange(g * H + r * 128, g * H + (r + 1) * 128)
             for g in range(4)])
        lstm_w = _pack_w(np.ascontiguousarray(w_cat[:, cols]))  # [128,16*512]
        lstm_b = b_cat[cols].reshape(1, 512)
        wv = out_WT_pad[:, r * VS:(r + 1) * VS]           # [1024, VS]
        wout = np.ascontiguousarray(
            wv.reshape(8, 128, NVT, 512).transpose(2, 1, 0, 3)
        ).reshape(NVT, 128, 8 * 512).astype(NPW)
        bias_all = np.concatenate([
            attn_bv.reshape(-1), comb_bv.reshape(-1), lstm_b.reshape(-1),
            c0v[r * 128:(r + 1) * 128]]).reshape(1, 2176).astype(f)
        in_maps.append({
            "cst_all": cst_all, "bias_all": bias_all,
            "attn_w": attn_w, "enc": enc, "comb_w": comb_w,
            "lstm_w": lstm_w, "wout": wout,
        })
    return in_maps, out_b_pad


def run_on_device(in_maps, trace=False):
    if "nc" not in _cache:
        _cache["nc"] = _build()
    nc = _cache["nc"]
    return bass_utils.run_bass_kernel_spmd(
        nc, in_maps, core_ids=list(range(NCORES)), trace=trace)


def kernel(**inputs):
    in_maps, out_b_pad = _prep_inputs(**inputs)
    res = run_on_device(in_maps).results

    logits = np.concatenate(
        [res[r]["logits_o"].reshape(VS) for r in range(NCORES)])[:VOCAB]
    logits = logits + out_b_pad[:VOCAB]
    m = float(logits.max())
    lse = m + float(np.log(np.exp(logits - m, dtype=np.float64).sum()))
    out = (logits - np.float32(lse)).astype(np.float32).reshape(1, VOCAB)

    h = np.concatenate([res[r]["h_o"].reshape(128) for r in range(NCORES)])
    c = np.concatenate([res[r]["c_o"].reshape(128) for r in range(NCORES)])
    attnw = res[0]["attnw_o"].reshape(1, SEQ).astype(np.float32)
    return (out,
            h.astype(np.float32).reshape(1, 1, H),
            c.astype(np.float32).reshape(1, 1, H),
            attnw)


# revision 14
# speedup vs baseline: 1.4932x; 1.3475x over previous
"""AttnDecoderRNN single-step on 8 trn2 NeuronCores.

Strategy (tensor parallel over vocab, per sharding hint):
- Embedding lookup on host (only the one needed row of the 206MB table is
  ever read; shipping the table to the device would be pure waste).
- Attention + combine replicated on every core (small weights).
- LSTM cell sharded over the hidden dim: core r computes gate slices
  i/f/g/o[r*128:(r+1)*128] -> its h/c slice.
- AllGather of the 128-elem h slices -> full h on every core.
- Output projection sharded over vocab: core r computes logits for its
  6656 (padded) vocab columns. log_softmax normalization is finished on
  host (combines 8 shard stats; O(vocab) host work).

All matvecs run the activation vector as the PE stationary operand and
stream the weight matrix as the moving operand. Weights are host-packed
into the exact SBUF layout and streamed in [128, 4096] chunks through one
tile pool, so weight DMA pipelines with PE and the out-projection panels
prefetch during phase 1. Vector transposes ([1,n*128] -> [128,n]) are done
on the PE via K=1 matmuls against a ones scalar (no DRAM bounce).
"""

import os
import numpy as np
import ml_dtypes

import concourse.bass as bass
import concourse.bacc as bacc
import concourse.mybir as mybir
import concourse.tile as tile
from concourse import bass_utils

NCORES = 8
H = 1024
SEQ = 512
VOCAB = 50257
VS = 6656               # padded per-core vocab shard; 8*VS = 53248
NVT = VS // 512         # 13 weight panels of [128, 8*512] per core
CHUNK = 4096            # streaming chunk free-dim (8KB/partition bf16)
KD = os.environ.get("KDTYPE", "bf16")
NPW = ml_dtypes.bfloat16 if KD == "bf16" else np.float32
WBUFS = int(os.environ.get("KWBUFS", "21"))
KOUT8 = os.environ.get("KOUT8", "1") == "1"   # fp8 e4m3 out-projection stream
NPO = ml_dtypes.float8_e4m3 if KOUT8 else NPW

_cache = {}


def _pack_kxm(vec):
    """[C*128] vector -> [128, C] chunk-per-column layout (PE stationary)."""
    c = vec.shape[0] // 128
    return np.ascontiguousarray(vec.reshape(c, 128).T).astype(NPW)


def _pack_w(wT):
    """[K, N] weight (K = contraction, mult of 128) -> [128, (K/128)*N];
    column block k holds rows k*128:(k+1)*128."""
    k = wT.shape[0] // 128
    n = wT.shape[1]
    return np.ascontiguousarray(
        wT.reshape(k, 128, n).transpose(1, 0, 2).reshape(128, k * n)
    ).astype(NPW)


def _build(ablate=()):
    wdt = mybir.dt.bfloat16 if KD == "bf16" else mybir.dt.float32
    odt = mybir.dt.float8e4 if KOUT8 else wdt
    f32 = mybir.dt.float32
    AF = mybir.ActivationFunctionType
    OP = mybir.AluOpType

    nc = bacc.Bacc("TRN2", target_bir_lowering=False, debug=False,
                   num_devices=NCORES)

    def din(name, shape, dt=None):
        return nc.dram_tensor(name, shape, dt or wdt, kind="ExternalInput").ap()

    def dout(name, shape):
        return nc.dram_tensor(name, shape, f32, kind="ExternalOutput").ap()

    # cst_all: cols 0:16 = ain kxm, col 16 = ones, cols 17:25 rows 0:8 = eye8
    d_cst = din("cst_all", [128, 32])
    # bias_all: 0:512 attn_b | 512:1536 comb_b | 1536:2048 lstm_b | 2048:2176 c0
    d_bias = din("bias_all", [1, 2176], f32)
    d_attnw = din("attn_w", [128, 16 * 512])      # 2 chunks
    d_enc = din("enc", [128, 4 * 1024])           # 1 chunk
    d_combw = din("comb_w", [128, 16 * 1024])     # 4 chunks
    d_lstmw = din("lstm_w", [128, 16 * 512])      # 2 chunks
    d_wout = din("wout", [NVT, 128, 8 * 512], odt)   # 13 panels

    d_logits = dout("logits_o", [1, VS])
    d_ho = dout("h_o", [1, 128])
    d_co = dout("c_o", [1, 128])
    d_awo = dout("attnw_o", [1, 512])

    with tile.TileContext(nc) as tc:
        with (
            tc.tile_pool(name="const", bufs=1) as cpool,
            tc.tile_pool(name="wpool", bufs=WBUFS) as wpool,
            tc.tile_pool(name="act", bufs=1) as apool,
            tc.tile_pool(name="ps", bufs=2, space="PSUM") as pspool,
            tc.tile_pool(name="pst", bufs=2, space="PSUM") as pstpool,
            tc.tile_pool(name="pso", bufs=2, space="PSUM") as psopool,
            tc.tile_pool(name="dram", bufs=1, space="DRAM") as dpool,
        ):
            # ---- two merged constant loads (unblock phase-1 chain fast)
            cst = cpool.tile([128, 32], wdt)
            nc.scalar.dma_start(cst[:], d_cst[:])
            bias = cpool.tile([1, 2176], f32)
            nc.scalar.dma_start(bias[:], d_bias[:])
            ain = cst[:, 0:16]
            one = cst[0:1, 16:17]
            eye8 = cst[0:8, 17:25]
            ab = bias[:, 0:512]
            cb = bias[:, 512:1536]
            lb = bias[:, 1536:2048]
            c0s = bias[:, 2048:2176]

            def wchunk(src_ap, name, dt=None):
                t = wpool.tile([128, CHUNK], dt or wdt, name=name, tag="ws")
                nc.sync.dma_start(t[:], src_ap)
                return t

            # ---- streamed weight chunks, program order = DMA priority
            aw0 = wchunk(d_attnw[:, 0:CHUNK], "aw0")
            aw1 = wchunk(d_attnw[:, CHUNK:2 * CHUNK], "aw1")
            enc = wchunk(d_enc[:], "enc")
            cwc = [wchunk(d_combw[:, c * CHUNK:(c + 1) * CHUNK], f"cw{c}")
                   for c in range(4)]
            lw0 = wchunk(d_lstmw[:, 0:CHUNK], "lw0")
            lw1 = wchunk(d_lstmw[:, CHUNK:2 * CHUNK], "lw1")
            NPRE = int(os.environ.get("KNPRE", "5"))
            wps = [wchunk(d_wout[v], f"wp{v}", odt) for v in range(NPRE)]

            def vec_to_kxm(src_f32_ap, n, name):
                """[1, n*128] f32 -> [128, n] wdt via K=1 PE matmuls."""
                vc = apool.tile([1, n * 128], wdt, name=name + "_c")
                nc.vector.tensor_copy(vc[:], src_f32_ap)
                pst = pstpool.tile([128, 8], f32, name=name + "_p", tag="pst")
                for c in range(n):
                    nc.tensor.matmul(pst[:, c:c + 1],
                                     vc[:, c * 128:(c + 1) * 128], one,
                                     start=True, stop=True)
                kxm = apool.tile([128, n], wdt, name=name + "_k")
                nc.vector.tensor_copy(kxm[:], pst[:, 0:n])
                return kxm

            # ---------- stage 1: attention scores + softmax [1,512]
            ps_s = pspool.tile([1, 1024], f32, name="ps_s", tag="ps")
            for k in range(16):
                src = (aw0 if k < 8 else aw1)
                kk = k % 8
                nc.tensor.matmul(ps_s[:, 0:512], ain[:, k:k + 1],
                                 src[:, kk * 512:(kk + 1) * 512],
                                 start=(k == 0), stop=(k == 15))
            scores = apool.tile([1, 512], f32)
            nc.vector.tensor_add(scores[:], ps_s[:, 0:512], ab)
            mx = apool.tile([1, 1], f32)
            nc.vector.reduce_max(mx[:], scores[:], axis=mybir.AxisListType.X)
            nmx = apool.tile([1, 1], f32)
            nc.vector.tensor_scalar_mul(nmx[:], mx[:], -1.0)
            expv = apool.tile([1, 512], f32)
            ssum = apool.tile([1, 1], f32)
            nc.scalar.activation(expv[:], scores[:], AF.Exp, bias=nmx[:],
                                 scale=1.0, accum_out=ssum[:])
            rs = apool.tile([1, 1], f32)
            nc.vector.reciprocal(rs[:], ssum[:])
            awf = apool.tile([1, 512], f32)
            nc.vector.tensor_scalar_mul(awf[:], expv[:], rs[:])
            aw_kxm = vec_to_kxm(awf[:], 4, "awr")

            # ---------- stage 2: attn_applied [1,1024]
            ps_a = pspool.tile([1, 1024], f32, name="ps_a", tag="ps")
            for n in range(2):
                for k in range(4):
                    nc.tensor.matmul(
                        ps_a[:, n * 512:(n + 1) * 512], aw_kxm[:, k:k + 1],
                        enc[:, k * 1024 + n * 512: k * 1024 + (n + 1) * 512],
                        start=(k == 0), stop=(k == 3))
            aa_kxm = vec_to_kxm(ps_a[:], 8, "aar")

            # ---------- stage 3: combine + relu -> x [1,1024]
            ps_x = pspool.tile([1, 1024], f32, name="ps_x", tag="ps")
            for n in range(2):
                for k in range(16):
                    lhs = ain[:, k:k + 1] if k < 8 else aa_kxm[:, k - 8:k - 7]
                    c, kk = divmod(k * 1024 + n * 512, CHUNK)
                    nc.tensor.matmul(
                        ps_x[:, n * 512:(n + 1) * 512], lhs,
                        cwc[c][:, kk:kk + 512],
                        start=(k == 0), stop=(k == 15))
            xb = apool.tile([1, 1024], f32)
            nc.vector.tensor_add(xb[:], ps_x[:], cb)
            xr = apool.tile([1, 1024], f32)
            nc.vector.tensor_scalar_max(xr[:], xb[:], 0.0)
            x_kxm = vec_to_kxm(xr[:], 8, "xr")

            # ---------- stage 4: LSTM gate slices [1,512] = [i|f|g|o]x128
            ps_g = pspool.tile([1, 1024], f32, name="ps_g", tag="ps")
            for k in range(16):
                lhs = x_kxm[:, k:k + 1] if k < 8 else ain[:, k:k + 1]
                src = lw0 if k < 8 else lw1
                kk = k % 8
                nc.tensor.matmul(ps_g[:, 0:512], lhs,
                                 src[:, kk * 512:(kk + 1) * 512],
                                 start=(k == 0), stop=(k == 15))
            gt = apool.tile([1, 512], f32)
            nc.vector.tensor_add(gt[:], ps_g[:, 0:512], lb)
            acts = apool.tile([1, 512], f32)
            nc.scalar.activation(acts[:, 0:256], gt[:, 0:256], AF.Sigmoid)
            nc.scalar.activation(acts[:, 256:384], gt[:, 256:384], AF.Tanh)
            nc.scalar.activation(acts[:, 384:512], gt[:, 384:512], AF.Sigmoid)
            fc = apool.tile([1, 128], f32)
            nc.vector.tensor_mul(fc[:], acts[:, 128:256], c0s)
            ig = apool.tile([1, 128], f32)
            nc.vector.tensor_mul(ig[:], acts[:, 0:128], acts[:, 256:384])
            cn = apool.tile([1, 128], f32)
            nc.vector.tensor_add(cn[:], fc[:], ig[:])
            tch = apool.tile([1, 128], f32)
            nc.scalar.activation(tch[:], cn[:], AF.Tanh)
            hn = apool.tile([1, 128], f32)
            hn_inst = nc.vector.tensor_mul(hn[:], acts[:, 384:512], tch[:])
            for v in range(NPRE, NVT):
                t = wpool.tile([128, CHUNK], odt, name=f"wp{v}", tag="ws")
                dls = nc.sync.dma_start(t[:], d_wout[v])
                tile.add_dep_helper(getattr(dls, "ins", dls),
                                    getattr(hn_inst, "ins", hn_inst), sync=True)
                wps.append(t)

            # ---------- AllGather h slices -> full h, transpose to [128, 8]
            hc = apool.tile([1, 128], wdt)
            nc.vector.tensor_copy(hc[:], hn[:])
            ag_in = dpool.tile([1, 128], wdt)
            ag_out = dpool.tile([8, 128], wdt, addr_space="Shared")
            nc.gpsimd.dma_start(ag_in[:], hc[:])
            if "noag" not in ablate:
                nc.gpsimd.collective_compute(
                    "AllGather", OP.bypass,
                    replica_groups=[list(range(NCORES))],
                    ins=[ag_in[:]], outs=[ag_out[:]])
            hsb = apool.tile([8, 128], wdt)
            nc.gpsimd.dma_start(hsb[:], ag_out[:])
            ps_h = pstpool.tile([128, 8], f32, name="ps_h", tag="pst")
            nc.tensor.matmul(ps_h[:], hsb[:], eye8, start=True, stop=True)
            h_kxm = apool.tile([128, 8], odt)
            nc.vector.tensor_copy(h_kxm[:], ps_h[:])

            nc.gpsimd.dma_start(d_awo[:], awf[:])
            nc.gpsimd.dma_start(d_co[:], cn[:])
            nc.gpsimd.dma_start(d_ho[:], hn[:])

            # ---------- stage 5: vocab-sharded output projection
            # (out_b is added on host with the shard-combining log_softmax)
            for v in range(NVT):
                ps_o = psopool.tile([1, 512], f32, name="ps_o", tag="pso")
                for k in range(8):
                    nc.tensor.matmul(ps_o[:], h_kxm[:, k:k + 1],
                                     wps[v][:, k * 512:(k + 1) * 512],
                                     start=(k == 0), stop=(k == 7))
                lst = apool.tile([1, 512], f32, name="lst", tag="lst", bufs=2)
                nc.vector.tensor_copy(lst[:], ps_o[:])
                nc.scalar.dma_start(d_logits[:, v * 512:(v + 1) * 512], lst[:])

    nc.compile()
    return nc


def _prep_inputs(input_idx, h0, c0, encoder_outputs, emb, attn_W, attn_b,
                 comb_W, comb_b, w_ih, w_hh, b_ih, b_hh, out_W, out_b):
    f = np.float32
    idx = int(np.asarray(input_idx).reshape(-1)[0])
    embedded = np.asarray(emb, f)[idx]                    # [H]
    h0v = np.asarray(h0, f).reshape(H)
    c0v = np.asarray(c0, f).reshape(H)

    ain = _pack_kxm(np.concatenate([embedded, h0v]))      # [128, 16]
    attn_w = _pack_w(np.asarray(attn_W, f).T)             # [128, 16*512]
    enc = _pack_w(np.asarray(encoder_outputs, f))         # [128, 4*1024]
    comb_w = _pack_w(np.asarray(comb_W, f).T)             # [128, 16*1024]
    attn_bv = np.asarray(attn_b, f).reshape(1, 512)
    comb_bv = np.asarray(comb_b, f).reshape(1, 1024)

    w_cat = np.concatenate([np.asarray(w_ih, f).T,
                            np.asarray(w_hh, f).T], axis=0)   # [2048, 4096]
    b_cat = (np.asarray(b_ih, f) + np.asarray(b_hh, f))       # [4096]

    out_WT = np.asarray(out_W, f).T                       # [1024, VOCAB]
    out_WT_pad = np.zeros((H, NCORES * VS), f)
    out_WT_pad[:, :VOCAB] = out_WT
    out_b_pad = np.zeros(NCORES * VS, f)
    out_b_pad[:VOCAB] = np.asarray(out_b, f)

    cst_all = np.zeros((128, 32), f)
    cst_all[:, 0:16] = ain.astype(f)
    cst_all[0, 16] = 1.0
    cst_all[0:8, 17:25] = np.eye(8, dtype=f)
    cst_all = cst_all.astype(NPW)

    in_maps = []
    for r in range(NCORES):
        cols = np.concatenate(
            [np.arange(g * H + r * 128, g * H + (r + 1) * 128)
             for g in range(4)])
        lstm_w = _pack_w(np.ascontiguousarray(w_cat[:, cols]))  # [128,16*512]
        lstm_b = b_cat[cols].reshape(1, 512)
        wv = out_WT_pad[:, r * VS:(r + 1) * VS]           # [1024, VS]
        wout = np.ascontiguousarray(
            wv.reshape(8, 128, NVT, 512).transpose(2, 1, 0, 3)
        ).reshape(NVT, 128, 8 * 512).astype(NPO)
        bias_all = np.concatenate([
            attn_bv.reshape(-1), comb_bv.reshape(-1), lstm_b.reshape(-1),
            c0v[r * 128:(r + 1) * 128]]).reshape(1, 2176).astype(f)
        in_maps.append({
            "cst_all": cst_all, "bias_all": bias_all,
            "attn_w": attn_w, "enc": enc, "comb_w": comb_w,
            "lstm_w": lstm_w, "wout": wout,
        })
    return in_maps, out_b_pad


def run_on_device(in_maps, trace=False):
    if "nc" not in _cache:
        _cache["nc"] = _build()
    nc = _cache["nc"]
    return bass_utils.run_bass_kernel_spmd(
        nc, in_maps, core_ids=list(range(NCORES)), trace=trace)


def kernel(**inputs):
    in_maps, out_b_pad = _prep_inputs(**inputs)
    res = run_on_device(in_maps).results

    logits = np.concatenate(
        [res[r]["logits_o"].reshape(VS) for r in range(NCORES)])[:VOCAB]
    logits = logits + out_b_pad[:VOCAB]
    m = float(logits.max())
    lse = m + float(np.log(np.exp(logits - m, dtype=np.float64).sum()))
    out = (logits - np.float32(lse)).astype(np.float32).reshape(1, VOCAB)

    h = np.concatenate([res[r]["h_o"].reshape(128) for r in range(NCORES)])
    c = np.concatenate([res[r]["c_o"].reshape(128) for r in range(NCORES)])
    attnw = res[0]["attnw_o"].reshape(1, SEQ).astype(np.float32)
    return (out,
            h.astype(np.float32).reshape(1, 1, H),
            c.astype(np.float32).reshape(1, 1, H),
            attnw)


# revision 17
# speedup vs baseline: 1612.7862x; 1080.1112x over previous
"""AttnDecoderRNN single-step on 8 trn2 NeuronCores.

Strategy (tensor parallel over vocab, per sharding hint):
- Embedding lookup on host (only the one needed row of the 206MB table is
  ever read; shipping the table to the device would be pure waste).
- Attention + combine replicated on every core (small weights).
- LSTM cell sharded over the hidden dim: core r computes gate slices
  i/f/g/o[r*128:(r+1)*128] -> its h/c slice.
- AllGather of the 128-elem h slices -> full h on every core.
- Output projection sharded over vocab: core r computes logits for its
  6656 (padded) vocab columns. log_softmax normalization is finished on
  host (combines 8 shard stats; O(vocab) host work).

All matvecs run the activation vector as the PE stationary operand and
stream the weight matrix as the moving operand. Weights are host-packed
into the exact SBUF layout and streamed in [128, 4096] chunks through one
tile pool, so weight DMA pipelines with PE and the out-projection panels
prefetch during phase 1. Vector transposes ([1,n*128] -> [128,n]) are done
on the PE via K=1 matmuls against a ones scalar (no DRAM bounce).
"""

import os
import numpy as np
import ml_dtypes

import concourse.bass as bass
import concourse.bacc as bacc
import concourse.mybir as mybir
import concourse.tile as tile
from concourse import bass_utils

NCORES = 8
H = 1024
SEQ = 512
VOCAB = 50257
VS = 6656               # padded per-core vocab shard; 8*VS = 53248
NVT = VS // 512         # 13 weight panels of [128, 8*512] per core
CHUNK = 4096            # streaming chunk free-dim (8KB/partition bf16)
KD = os.environ.get("KDTYPE", "bf16")
NPW = ml_dtypes.bfloat16 if KD == "bf16" else np.float32
WBUFS = int(os.environ.get("KWBUFS", "9"))
KOUT8 = os.environ.get("KOUT8", "1") == "1"   # fp8 e4m3 out-projection stream
KPACK = os.environ.get("KPACK", "1") == "1"   # 4-wide PE col-group packing
NPO = ml_dtypes.float8_e4m3 if KOUT8 else NPW

_cache = {}


def _pack_kxm(vec):
    """[C*128] vector -> [128, C] chunk-per-column layout (PE stationary)."""
    c = vec.shape[0] // 128
    return np.ascontiguousarray(vec.reshape(c, 128).T).astype(NPW)


def _pack_w(wT):
    """[K, N] weight (K = contraction, mult of 128) -> [128, (K/128)*N];
    column block k holds rows k*128:(k+1)*128."""
    k = wT.shape[0] // 128
    n = wT.shape[1]
    return np.ascontiguousarray(
        wT.reshape(k, 128, n).transpose(1, 0, 2).reshape(128, k * n)
    ).astype(NPW)


def _build(ablate=()):
    wdt = mybir.dt.bfloat16 if KD == "bf16" else mybir.dt.float32
    odt = mybir.dt.float8e4 if KOUT8 else wdt
    f32 = mybir.dt.float32
    AF = mybir.ActivationFunctionType
    OP = mybir.AluOpType

    nc = bacc.Bacc("TRN2", target_bir_lowering=False, debug=False,
                   num_devices=NCORES)

    def din(name, shape, dt=None):
        return nc.dram_tensor(name, shape, dt or wdt, kind="ExternalInput").ap()

    def dout(name, shape):
        return nc.dram_tensor(name, shape, f32, kind="ExternalOutput").ap()

    # cst_all: cols 0:16 = ain kxm, col 16 = ones, cols 17:25 rows 0:8 = eye8
    d_cst = din("cst_all", [128, 32])
    # bias_all: 0:512 attn_b | 512:1536 comb_b | 1536:2048 lstm_b | 2048:2176 c0
    d_bias = din("bias_all", [1, 2176], f32)
    d_attnw = din("attn_w", [128, 16 * 512])      # 2 chunks
    d_enc = din("enc", [128, 4 * 1024])           # 1 chunk
    d_combw = din("comb_w", [128, 16 * 1024])     # 4 chunks
    d_lstmw = din("lstm_w", [128, 16 * 512])      # 2 chunks
    d_wout = din("wout", [NVT, 128, 8 * 512], odt)   # 13 panels

    d_logits = dout("logits_o", [1, VS])
    d_ho = dout("h_o", [1, 128])
    d_co = dout("c_o", [1, 128])
    d_awo = dout("attnw_o", [1, 512])

    with tile.TileContext(nc) as tc:
        with (
            tc.tile_pool(name="const", bufs=1) as cpool,
            tc.tile_pool(name="wpool", bufs=WBUFS) as wpool,
            tc.tile_pool(name="act", bufs=1) as apool,
            tc.tile_pool(name="ps", bufs=2, space="PSUM") as pspool,
            tc.tile_pool(name="pst", bufs=2, space="PSUM") as pstpool,
            tc.tile_pool(name="pso", bufs=2, space="PSUM") as psopool,
            tc.tile_pool(name="dram", bufs=1, space="DRAM") as dpool,
        ):
            # ---- two merged constant loads (unblock phase-1 chain fast)
            cst = cpool.tile([128, 32], wdt)
            nc.scalar.dma_start(cst[:], d_cst[:])
            bias = cpool.tile([1, 2176], f32)
            nc.scalar.dma_start(bias[:], d_bias[:])
            ain = cst[:, 0:16]
            one = cst[0:1, 16:17]
            eye8 = cst[0:8, 17:25]
            ab = bias[:, 0:512]
            cb = bias[:, 512:1536]
            lb = bias[:, 1536:2048]
            c0s = bias[:, 2048:2176]

            def wchunk(src_ap, name, dt=None, tag="ws", bufs=9):
                t = wpool.tile([128, CHUNK], dt or wdt, name=name, tag=tag,
                               bufs=bufs)
                nc.sync.dma_start(t[:], src_ap)
                return t

            # ---- streamed weight chunks, program order = DMA priority
            aw0 = wchunk(d_attnw[:, 0:CHUNK], "aw0")
            aw1 = wchunk(d_attnw[:, CHUNK:2 * CHUNK], "aw1")
            enc = wchunk(d_enc[:], "enc")
            cwc = [wchunk(d_combw[:, c * CHUNK:(c + 1) * CHUNK], f"cw{c}")
                   for c in range(4)]
            lw0 = wchunk(d_lstmw[:, 0:CHUNK], "lw0")
            lw1 = wchunk(d_lstmw[:, CHUNK:2 * CHUNK], "lw1")
            NPRE = int(os.environ.get("KNPRE", "5"))
            wps = [wchunk(d_wout[v], f"wp{v}", odt, "wo", NVT)
                   for v in range(NPRE)]

            def vec_to_kxm(src_f32_ap, n, name):
                """[1, n*128] f32 -> [128, n] wdt via K=1 PE matmuls."""
                vc = apool.tile([1, n * 128], wdt, name=name + "_c")
                nc.vector.tensor_copy(vc[:], src_f32_ap)
                pst = pstpool.tile([128, 8], f32, name=name + "_p", tag="pst")
                for c in range(n):
                    nc.tensor.matmul(pst[:, c:c + 1],
                                     vc[:, c * 128:(c + 1) * 128], one,
                                     start=True, stop=True)
                kxm = apool.tile([128, n], wdt, name=name + "_k")
                nc.vector.tensor_copy(kxm[:], pst[:, 0:n])
                return kxm

            def packed_matvec(ps, nsl, lhs_of, rhs_of, nk, name):
                """sum_k lhs[k].T @ rhs[k] -> [1, 512] into sbuf tile.
                ps: [128, >=nsl*512] psum tile; 4 col groups accumulate
                in parallel rounds; partial rows 0/32/64/96 summed on DVE."""
                outs = []
                for n in range(nsl):
                    sl = slice(n * 512, (n + 1) * 512)
                    if KPACK and nk >= 8:
                        nr = (nk + 3) // 4
                        for r in range(nr):
                            for j in range(4):
                                k = r * 4 + j
                                if k >= nk:
                                    break
                                nc.tensor.matmul(
                                    ps[32 * j:32 * j + 1, sl], lhs_of(k),
                                    rhs_of(k, n), start=(r == 0),
                                    stop=(r >= nr - 1),
                                    tile_position=(0, 32 * j))
                        o = apool.tile([1, 512], f32, name=f"{name}o_{n}",
                                       tag=f"{name}o")
                        nc.vector.tensor_copy(o[:], ps[0:1, sl])
                        for j in range(1, 4):
                            nc.vector.tensor_add(o[:], o[:],
                                                 ps[32 * j:32 * j + 1, sl])
                        outs.append(o)
                    else:
                        for k in range(nk):
                            nc.tensor.matmul(ps[0:1, sl], lhs_of(k),
                                             rhs_of(k, n), start=(k == 0),
                                             stop=(k == nk - 1))
                        o = apool.tile([1, 512], f32, name=f"{name}o_{n}",
                                       tag=f"{name}o")
                        nc.vector.tensor_copy(o[:], ps[0:1, sl])
                        outs.append(o)
                return outs

            # ---------- stage 1: attention scores + softmax [1,512]
            ps_s = pspool.tile([128, 1024], f32, name="ps_s", tag="ps")
            (s_o,) = packed_matvec(
                ps_s, 1,
                lambda k: ain[:, k:k + 1],
                lambda k, n: (aw0 if k < 8 else aw1)[:, (k % 8) * 512:
                                                    (k % 8 + 1) * 512],
                16, "sc")
            scores = apool.tile([1, 512], f32)
            nc.vector.tensor_add(scores[:], s_o[:], ab)
            mx = apool.tile([1, 1], f32)
            nc.vector.reduce_max(mx[:], scores[:], axis=mybir.AxisListType.X)
            nmx = apool.tile([1, 1], f32)
            nc.vector.tensor_scalar_mul(nmx[:], mx[:], -1.0)
            expv = apool.tile([1, 512], f32)
            ssum = apool.tile([1, 1], f32)
            nc.scalar.activation(expv[:], scores[:], AF.Exp, bias=nmx[:],
                                 scale=1.0, accum_out=ssum[:])
            rs = apool.tile([1, 1], f32)
            nc.vector.reciprocal(rs[:], ssum[:])
            awf = apool.tile([1, 512], f32)
            nc.vector.tensor_scalar_mul(awf[:], expv[:], rs[:])
            aw_kxm = vec_to_kxm(awf[:], 4, "awr")

            # ---------- stage 2: attn_applied [1,1024]
            ps_a = pspool.tile([128, 1024], f32, name="ps_a", tag="ps")
            a_os = packed_matvec(
                ps_a, 2,
                lambda k: aw_kxm[:, k:k + 1],
                lambda k, n: enc[:, k * 1024 + n * 512:
                                 k * 1024 + (n + 1) * 512],
                4, "aa")
            aav = apool.tile([1, 1024], f32)
            nc.vector.tensor_copy(aav[:, 0:512], a_os[0][:])
            nc.vector.tensor_copy(aav[:, 512:1024], a_os[1][:])
            aa_kxm = vec_to_kxm(aav[:], 8, "aar")

            # ---------- stage 3: combine + relu -> x [1,1024]
            ps_x = pspool.tile([128, 1024], f32, name="ps_x", tag="ps")

            def cw_rhs(k, n):
                c, kk = divmod(k * 1024 + n * 512, CHUNK)
                return cwc[c][:, kk:kk + 512]

            x_os = packed_matvec(
                ps_x, 2,
                lambda k: ain[:, k:k + 1] if k < 8 else aa_kxm[:, k - 8:k - 7],
                cw_rhs, 16, "xx")
            xb = apool.tile([1, 1024], f32)
            nc.vector.tensor_add(xb[:, 0:512], x_os[0][:], cb[:, 0:512])
            nc.vector.tensor_add(xb[:, 512:1024], x_os[1][:], cb[:, 512:1024])
            xr = apool.tile([1, 1024], f32)
            nc.vector.tensor_scalar_max(xr[:], xb[:], 0.0)
            x_kxm = vec_to_kxm(xr[:], 8, "xr")

            # ---------- stage 4: LSTM gate slices [1,512] = [i|f|g|o]x128
            ps_g = pspool.tile([128, 1024], f32, name="ps_g", tag="ps")
            (g_o,) = packed_matvec(
                ps_g, 1,
                lambda k: x_kxm[:, k:k + 1] if k < 8 else ain[:, k:k + 1],
                lambda k, n: (lw0 if k < 8 else lw1)[:, (k % 8) * 512:
                                                    (k % 8 + 1) * 512],
                16, "gg")
            gt = apool.tile([1, 512], f32)
            nc.vector.tensor_add(gt[:], g_o[:], lb)
            acts = apool.tile([1, 512], f32)
            nc.scalar.activation(acts[:, 0:256], gt[:, 0:256], AF.Sigmoid)
            nc.scalar.activation(acts[:, 256:384], gt[:, 256:384], AF.Tanh)
            nc.scalar.activation(acts[:, 384:512], gt[:, 384:512], AF.Sigmoid)
            fc = apool.tile([1, 128], f32)
            nc.vector.tensor_mul(fc[:], acts[:, 128:256], c0s)
            ig = apool.tile([1, 128], f32)
            nc.vector.tensor_mul(ig[:], acts[:, 0:128], acts[:, 256:384])
            cn = apool.tile([1, 128], f32)
            nc.vector.tensor_add(cn[:], fc[:], ig[:])
            tch = apool.tile([1, 128], f32)
            nc.scalar.activation(tch[:], cn[:], AF.Tanh)
            hn = apool.tile([1, 128], f32)
            hn_inst = nc.vector.tensor_mul(hn[:], acts[:, 384:512], tch[:])
            for v in range(NPRE, NVT):
                t = wpool.tile([128, CHUNK], odt, name=f"wp{v}", tag="wo",
                               bufs=NVT)
                dls = nc.sync.dma_start(t[:], d_wout[v])
                tile.add_dep_helper(getattr(dls, "ins", dls),
                                    getattr(hn_inst, "ins", hn_inst), sync=True)
                wps.append(t)

            # ---------- AllGather h slices -> full h, transpose to [128, 8]
            hc = apool.tile([1, 128], wdt)
            nc.vector.tensor_copy(hc[:], hn[:])
            ag_in = dpool.tile([1, 128], wdt)
            ag_out = dpool.tile([8, 128], wdt, addr_space="Shared")
            nc.gpsimd.dma_start(ag_in[:], hc[:])
            if "noag" not in ablate:
                nc.gpsimd.collective_compute(
                    "AllGather", OP.bypass,
                    replica_groups=[list(range(NCORES))],
                    ins=[ag_in[:]], outs=[ag_out[:]])
            hsb = apool.tile([8, 128], wdt)
            nc.gpsimd.dma_start(hsb[:], ag_out[:])
            ps_h = pstpool.tile([128, 8], f32, name="ps_h", tag="pst")
            nc.tensor.matmul(ps_h[:], hsb[:], eye8, start=True, stop=True)
            h_kxm = apool.tile([128, 8], odt)
            nc.vector.tensor_copy(h_kxm[:], ps_h[:])

            nc.gpsimd.dma_start(d_awo[:], awf[:])
            nc.gpsimd.dma_start(d_co[:], cn[:])
            nc.gpsimd.dma_start(d_ho[:], hn[:])

            # ---------- stage 5: vocab-sharded output projection
            # (out_b is added on host with the shard-combining log_softmax)
            for v in range(NVT):
                ps_o = psopool.tile([1, 512], f32, name="ps_o", tag="pso")
                for k in range(8):
                    nc.tensor.matmul(ps_o[:], h_kxm[:, k:k + 1],
                                     wps[v][:, k * 512:(k + 1) * 512],
                                     start=(k == 0), stop=(k == 7))
                lst = apool.tile([1, 512], f32, name="lst", tag="lst", bufs=2)
                nc.vector.tensor_copy(lst[:], ps_o[:])
                nc.scalar.dma_start(d_logits[:, v * 512:(v + 1) * 512], lst[:])

    nc.compile()
    return nc


def _prep_inputs(input_idx, h0, c0, encoder_outputs, emb, attn_W, attn_b,
                 comb_W, comb_b, w_ih, w_hh, b_ih, b_hh, out_W, out_b):
    f = np.float32
    idx = int(np.asarray(input_idx).reshape(-1)[0])
    embedded = np.asarray(emb, f)[idx]                    # [H]
    h0v = np.asarray(h0, f).reshape(H)
    c0v = np.asarray(c0, f).reshape(H)

    ain = _pack_kxm(np.concatenate([embedded, h0v]))      # [128, 16]
    attn_w = _pack_w(np.asarray(attn_W, f).T)             # [128, 16*512]
    enc = _pack_w(np.asarray(encoder_outputs, f))         # [128, 4*1024]
    comb_w = _pack_w(np.asarray(comb_W, f).T)             # [128, 16*1024]
    attn_bv = np.asarray(attn_b, f).reshape(1, 512)
    comb_bv = np.asarray(comb_b, f).reshape(1, 1024)

    w_cat = np.concatenate([np.asarray(w_ih, f).T,
                            np.asarray(w_hh, f).T], axis=0)   # [2048, 4096]
    b_cat = (np.asarray(b_ih, f) + np.asarray(b_hh, f))       # [4096]

    out_WT = np.asarray(out_W, f).T                       # [1024, VOCAB]
    out_WT_pad = np.zeros((H, NCORES * VS), f)
    out_WT_pad[:, :VOCAB] = out_WT
    out_b_pad = np.zeros(NCORES * VS, f)
    out_b_pad[:VOCAB] = np.asarray(out_b, f)

    cst_all = np.zeros((128, 32), f)
    cst_all[:, 0:16] = ain.astype(f)
    cst_all[0, 16] = 1.0
    cst_all[0:8, 17:25] = np.eye(8, dtype=f)
    cst_all = cst_all.astype(NPW)

    in_maps = []
    for r in range(NCORES):
        cols = np.concatenate(
            [np.arange(g * H + r * 128, g * H + (r + 1) * 128)
             for g in range(4)])
        lstm_w = _pack_w(np.ascontiguousarray(w_cat[:, cols]))  # [128,16*512]
        lstm_b = b_cat[cols].reshape(1, 512)
        wv = out_WT_pad[:, r * VS:(r + 1) * VS]           # [1024, VS]
        wout = np.ascontiguousarray(
            wv.reshape(8, 128, NVT, 512).transpose(2, 1, 0, 3)
        ).reshape(NVT, 128, 8 * 512).astype(NPO)
        bias_all = np.concatenate([
            attn_bv.reshape(-1), comb_bv.reshape(-1), lstm_b.reshape(-1),
            c0v[r * 128:(r + 1) * 128]]).reshape(1, 2176).astype(f)
        in_maps.append({
            "cst_all": cst_all, "bias_all": bias_all,
            "attn_w": attn_w, "enc": enc, "comb_w": comb_w,
            "lstm_w": lstm_w, "wout": wout,
        })
    return in_maps, out_b_pad


def run_on_device(in_maps, trace=False):
    if "nc" not in _cache:
        _cache["nc"] = _build()
    nc = _cache["nc"]
    return bass_utils.run_bass_kernel_spmd(
        nc, in_maps, core_ids=list(range(NCORES)), trace=trace)


def kernel(**inputs):
    in_maps, out_b_pad = _prep_inputs(**inputs)
    res = run_on_device(in_maps).results

    logits = np.concatenate(
        [res[r]["logits_o"].reshape(VS) for r in range(NCORES)])[:VOCAB]
    logits = logits + out_b_pad[:VOCAB]
    m = float(logits.max())
    lse = m + float(np.log(np.exp(logits - m, dtype=np.float64).sum()))
    out = (logits - np.float32(lse)).astype(np.float32).reshape(1, VOCAB)

    h = np.concatenate([res[r]["h_o"].reshape(128) for r in range(NCORES)])
    c = np.concatenate([res[r]["c_o"].reshape(128) for r in range(NCORES)])
    attnw = res[0]["attnw_o"].reshape(1, SEQ).astype(np.float32)
    return (out,
            h.astype(np.float32).reshape(1, 1, H),
            c.astype(np.float32).reshape(1, 1, H),
            attnw)


# revision 18
# speedup vs baseline: 2439.9868x; 1.5129x over previous
"""AttnDecoderRNN single-step on 8 trn2 NeuronCores.

Strategy (tensor parallel over vocab, per sharding hint):
- Embedding lookup on host (only the one needed row of the 206MB table is
  ever read; shipping the table to the device would be pure waste).
- Attention + combine replicated on every core (small weights).
- LSTM cell sharded over the hidden dim: core r computes gate slices
  i/f/g/o[r*128:(r+1)*128] -> its h/c slice.
- AllGather of the 128-elem h slices -> full h on every core.
- Output projection sharded over vocab: core r computes logits for its
  6656 (padded) vocab columns. log_softmax normalization is finished on
  host (combines 8 shard stats; O(vocab) host work).

All matvecs run the activation vector as the PE stationary operand and
stream the weight matrix as the moving operand, 4-wide across PE column
groups (tile_position) with a DVE partial-row sum. Weights are host-packed
into the exact SBUF layout and streamed in [128, 4096]-element chunks
through one tile pool so weight DMA pipelines with PE; the out-projection
panels (fp8 e4m3 -- the logits have ~100x error headroom vs the h/c/attnw
outputs, which stay bf16) are issued 5-up-front + 8-deferred-on-h so the
AllGather's tiny DMA is not queued behind the bulk stream. Vector
transposes ([1,n*128] -> [128,n]) are PE K=1 matmuls against a ones
scalar (no DRAM bounce). Small/critical DMAs ride the ACT-HWDGE and
Pool-SWDGE queues, never the SP bulk-stream queue.
"""

import os
import numpy as np
import ml_dtypes

import concourse.bass as bass
import concourse.bacc as bacc
import concourse.mybir as mybir
import concourse.tile as tile
from concourse import bass_utils

NCORES = 8
H = 1024
SEQ = 512
VOCAB = 50257
VS = 6656               # padded per-core vocab shard; 8*VS = 53248
NVT = VS // 512         # 13 weight panels of [128, 8*512] per core
CHUNK = 4096            # streaming chunk free-dim (8KB/partition bf16)
KD = os.environ.get("KDTYPE", "bf16")
NPW = ml_dtypes.bfloat16 if KD == "bf16" else np.float32
WBUFS = int(os.environ.get("KWBUFS", "9"))
KOUT8 = os.environ.get("KOUT8", "1") == "1"   # fp8 e4m3 out-projection stream
KPACK = os.environ.get("KPACK", "1") == "1"   # 4-wide PE col-group packing
NPO = ml_dtypes.float8_e4m3 if KOUT8 else NPW

_cache = {}


def _pack_kxm(vec):
    """[C*128] vector -> [128, C] chunk-per-column layout (PE stationary)."""
    c = vec.shape[0] // 128
    return np.ascontiguousarray(vec.reshape(c, 128).T).astype(NPW)


def _pack_w(wT):
    """[K, N] weight (K = contraction, mult of 128) -> [128, (K/128)*N];
    column block k holds rows k*128:(k+1)*128."""
    k = wT.shape[0] // 128
    n = wT.shape[1]
    return np.ascontiguousarray(
        wT.reshape(k, 128, n).transpose(1, 0, 2).reshape(128, k * n)
    ).astype(NPW)


def _build(ablate=()):
    wdt = mybir.dt.bfloat16 if KD == "bf16" else mybir.dt.float32
    odt = mybir.dt.float8e4 if KOUT8 else wdt
    f32 = mybir.dt.float32
    AF = mybir.ActivationFunctionType
    OP = mybir.AluOpType

    nc = bacc.Bacc("TRN2", target_bir_lowering=False, debug=False,
                   num_devices=NCORES)

    def din(name, shape, dt=None):
        return nc.dram_tensor(name, shape, dt or wdt, kind="ExternalInput").ap()

    def dout(name, shape):
        return nc.dram_tensor(name, shape, f32, kind="ExternalOutput").ap()

    # cst_all: cols 0:16 = ain kxm, col 16 = ones, cols 17:25 rows 0:8 = eye8
    d_cst = din("cst_all", [128, 32])
    # bias_all: 0:512 attn_b | 512:1536 comb_b | 1536:2048 lstm_b | 2048:2176 c0
    d_bias = din("bias_all", [1, 2176], f32)
    d_attnw = din("attn_w", [128, 16 * 512])      # 2 chunks
    d_enc = din("enc", [128, 4 * 1024])           # 1 chunk
    d_combw = din("comb_w", [128, 16 * 1024])     # 4 chunks
    d_lstmw = din("lstm_w", [128, 16 * 512])      # 2 chunks
    d_wout = din("wout", [NVT, 128, 8 * 512], odt)   # 13 panels

    d_logits = dout("logits_o", [1, VS])
    d_ho = dout("h_o", [1, 128])
    d_co = dout("c_o", [1, 128])
    d_awo = dout("attnw_o", [1, 512])

    with tile.TileContext(nc) as tc:
        with (
            tc.tile_pool(name="const", bufs=1) as cpool,
            tc.tile_pool(name="wpool", bufs=WBUFS) as wpool,
            tc.tile_pool(name="act", bufs=1) as apool,
            tc.tile_pool(name="ps", bufs=2, space="PSUM") as pspool,
            tc.tile_pool(name="pst", bufs=2, space="PSUM") as pstpool,
            tc.tile_pool(name="pso", bufs=2, space="PSUM") as psopool,
            tc.tile_pool(name="dram", bufs=1, space="DRAM") as dpool,
        ):
            # ---- two merged constant loads (unblock phase-1 chain fast)
            cst = cpool.tile([128, 32], wdt)
            nc.scalar.dma_start(cst[:], d_cst[:])
            bias = cpool.tile([1, 2176], f32)
            nc.scalar.dma_start(bias[:], d_bias[:])
            ain = cst[:, 0:16]
            one = cst[0:1, 16:17]
            eye8 = cst[0:8, 17:25]
            ab = bias[:, 0:512]
            cb = bias[:, 512:1536]
            lb = bias[:, 1536:2048]
            c0s = bias[:, 2048:2176]

            def wchunk(src_ap, name, dt=None, tag="ws", bufs=9):
                t = wpool.tile([128, CHUNK], dt or wdt, name=name, tag=tag,
                               bufs=bufs)
                nc.sync.dma_start(t[:], src_ap)
                return t

            # ---- streamed weight chunks, program order = DMA priority
            aw0 = wchunk(d_attnw[:, 0:CHUNK], "aw0")
            aw1 = wchunk(d_attnw[:, CHUNK:2 * CHUNK], "aw1")
            enc = wchunk(d_enc[:], "enc")
            cwc = [wchunk(d_combw[:, c * CHUNK:(c + 1) * CHUNK], f"cw{c}")
                   for c in range(4)]
            lw0 = wchunk(d_lstmw[:, 0:CHUNK], "lw0")
            lw1 = wchunk(d_lstmw[:, CHUNK:2 * CHUNK], "lw1")
            NPRE = int(os.environ.get("KNPRE", "5"))
            wps = [wchunk(d_wout[v], f"wp{v}", odt, "wo", NVT)
                   for v in range(NPRE)]

            def vec_to_kxm(src_f32_ap, n, name):
                """[1, n*128] f32 -> [128, n] wdt via K=1 PE matmuls."""
                vc = apool.tile([1, n * 128], wdt, name=name + "_c")
                nc.vector.tensor_copy(vc[:], src_f32_ap)
                pst = pstpool.tile([128, 8], f32, name=name + "_p", tag="pst")
                for c in range(n):
                    nc.tensor.matmul(pst[:, c:c + 1],
                                     vc[:, c * 128:(c + 1) * 128], one,
                                     start=True, stop=True)
                kxm = apool.tile([128, n], wdt, name=name + "_k")
                nc.vector.tensor_copy(kxm[:], pst[:, 0:n])
                return kxm

            def packed_matvec(ps, nsl, lhs_of, rhs_of, nk, name):
                """sum_k lhs[k].T @ rhs[k] -> [1, 512] into sbuf tile.
                ps: [128, >=nsl*512] psum tile; 4 col groups accumulate
                in parallel rounds; partial rows 0/32/64/96 summed on DVE."""
                outs = []
                for n in range(nsl):
                    sl = slice(n * 512, (n + 1) * 512)
                    if KPACK and nk >= 8:
                        nr = (nk + 3) // 4
                        for r in range(nr):
                            for j in range(4):
                                k = r * 4 + j
                                if k >= nk:
                                    break
                                nc.tensor.matmul(
                                    ps[32 * j:32 * j + 1, sl], lhs_of(k),
                                    rhs_of(k, n), start=(r == 0),
                                    stop=(r >= nr - 1),
                                    tile_position=(0, 32 * j))
                        o = apool.tile([1, 512], f32, name=f"{name}o_{n}",
                                       tag=f"{name}o")
                        nc.vector.tensor_copy(o[:], ps[0:1, sl])
                        for j in range(1, 4):
                            nc.vector.tensor_add(o[:], o[:],
                                                 ps[32 * j:32 * j + 1, sl])
                        outs.append(o)
                    else:
                        for k in range(nk):
                            nc.tensor.matmul(ps[0:1, sl], lhs_of(k),
                                             rhs_of(k, n), start=(k == 0),
                                             stop=(k == nk - 1))
                        o = apool.tile([1, 512], f32, name=f"{name}o_{n}",
                                       tag=f"{name}o")
                        nc.vector.tensor_copy(o[:], ps[0:1, sl])
                        outs.append(o)
                return outs

            # ---------- stage 1: attention scores + softmax [1,512]
            ps_s = pspool.tile([128, 1024], f32, name="ps_s", tag="ps")
            (s_o,) = packed_matvec(
                ps_s, 1,
                lambda k: ain[:, k:k + 1],
                lambda k, n: (aw0 if k < 8 else aw1)[:, (k % 8) * 512:
                                                    (k % 8 + 1) * 512],
                16, "sc")
            scores = apool.tile([1, 512], f32)
            nc.vector.tensor_add(scores[:], s_o[:], ab)
            mx = apool.tile([1, 1], f32)
            nc.vector.reduce_max(mx[:], scores[:], axis=mybir.AxisListType.X)
            nmx = apool.tile([1, 1], f32)
            nc.vector.tensor_scalar_mul(nmx[:], mx[:], -1.0)
            expv = apool.tile([1, 512], f32)
            ssum = apool.tile([1, 1], f32)
            nc.scalar.activation(expv[:], scores[:], AF.Exp, bias=nmx[:],
                                 scale=1.0, accum_out=ssum[:])
            rs = apool.tile([1, 1], f32)
            nc.vector.reciprocal(rs[:], ssum[:])
            awf = apool.tile([1, 512], f32)
            nc.vector.tensor_scalar_mul(awf[:], expv[:], rs[:])
            aw_kxm = vec_to_kxm(awf[:], 4, "awr")

            # ---------- stage 2: attn_applied [1,1024]
            ps_a = pspool.tile([128, 1024], f32, name="ps_a", tag="ps")
            a_os = packed_matvec(
                ps_a, 2,
                lambda k: aw_kxm[:, k:k + 1],
                lambda k, n: enc[:, k * 1024 + n * 512:
                                 k * 1024 + (n + 1) * 512],
                4, "aa")
            aav = apool.tile([1, 1024], f32)
            nc.vector.tensor_copy(aav[:, 0:512], a_os[0][:])
            nc.vector.tensor_copy(aav[:, 512:1024], a_os[1][:])
            aa_kxm = vec_to_kxm(aav[:], 8, "aar")

            # ---------- stage 3: combine + relu -> x [1,1024]
            ps_x = pspool.tile([128, 1024], f32, name="ps_x", tag="ps")

            def cw_rhs(k, n):
                c, kk = divmod(k * 1024 + n * 512, CHUNK)
                return cwc[c][:, kk:kk + 512]

            x_os = packed_matvec(
                ps_x, 2,
                lambda k: ain[:, k:k + 1] if k < 8 else aa_kxm[:, k - 8:k - 7],
                cw_rhs, 16, "xx")
            xb = apool.tile([1, 1024], f32)
            nc.vector.tensor_add(xb[:, 0:512], x_os[0][:], cb[:, 0:512])
            nc.vector.tensor_add(xb[:, 512:1024], x_os[1][:], cb[:, 512:1024])
            xr = apool.tile([1, 1024], f32)
            nc.vector.tensor_scalar_max(xr[:], xb[:], 0.0)
            x_kxm = vec_to_kxm(xr[:], 8, "xr")

            # ---------- stage 4: LSTM gate slices [1,512] = [i|f|g|o]x128
            ps_g = pspool.tile([128, 1024], f32, name="ps_g", tag="ps")
            (g_o,) = packed_matvec(
                ps_g, 1,
                lambda k: x_kxm[:, k:k + 1] if k < 8 else ain[:, k:k + 1],
                lambda k, n: (lw0 if k < 8 else lw1)[:, (k % 8) * 512:
                                                    (k % 8 + 1) * 512],
                16, "gg")
            gt = apool.tile([1, 512], f32)
            nc.vector.tensor_add(gt[:], g_o[:], lb)
            acts = apool.tile([1, 512], f32)
            nc.scalar.activation(acts[:, 0:256], gt[:, 0:256], AF.Sigmoid)
            nc.scalar.activation(acts[:, 256:384], gt[:, 256:384], AF.Tanh)
            nc.scalar.activation(acts[:, 384:512], gt[:, 384:512], AF.Sigmoid)
            fc = apool.tile([1, 128], f32)
            nc.vector.tensor_mul(fc[:], acts[:, 128:256], c0s)
            ig = apool.tile([1, 128], f32)
            nc.vector.tensor_mul(ig[:], acts[:, 0:128], acts[:, 256:384])
            cn = apool.tile([1, 128], f32)
            nc.vector.tensor_add(cn[:], fc[:], ig[:])
            tch = apool.tile([1, 128], f32)
            nc.scalar.activation(tch[:], cn[:], AF.Tanh)
            hn = apool.tile([1, 128], f32)
            hn_inst = nc.vector.tensor_mul(hn[:], acts[:, 384:512], tch[:])
            for v in range(NPRE, NVT):
                t = wpool.tile([128, CHUNK], odt, name=f"wp{v}", tag="wo",
                               bufs=NVT)
                dls = nc.sync.dma_start(t[:], d_wout[v])
                tile.add_dep_helper(getattr(dls, "ins", dls),
                                    getattr(hn_inst, "ins", hn_inst), sync=True)
                wps.append(t)

            # ---------- AllGather h slices -> full h, transpose to [128, 8]
            hc = apool.tile([1, 128], wdt)
            nc.vector.tensor_copy(hc[:], hn[:])
            ag_in = dpool.tile([1, 128], wdt)
            ag_out = dpool.tile([8, 128], wdt, addr_space="Shared")
            nc.gpsimd.dma_start(ag_in[:], hc[:])
            if "noag" not in ablate:
                nc.gpsimd.collective_compute(
                    "AllGather", OP.bypass,
                    replica_groups=[list(range(NCORES))],
                    ins=[ag_in[:]], outs=[ag_out[:]])
            hsb = apool.tile([8, 128], wdt)
            nc.gpsimd.dma_start(hsb[:], ag_out[:])
            ps_h = pstpool.tile([128, 8], f32, name="ps_h", tag="pst")
            nc.tensor.matmul(ps_h[:], hsb[:], eye8, start=True, stop=True)
            h_kxm = apool.tile([128, 8], odt)
            nc.vector.tensor_copy(h_kxm[:], ps_h[:])

            nc.gpsimd.dma_start(d_awo[:], awf[:])
            nc.gpsimd.dma_start(d_co[:], cn[:])
            nc.gpsimd.dma_start(d_ho[:], hn[:])

            # ---------- stage 5: vocab-sharded output projection
            # (out_b is added on host with the shard-combining log_softmax)
            for v in range(NVT):
                ps_o = psopool.tile([1, 512], f32, name="ps_o", tag="pso")
                for k in range(8):
                    nc.tensor.matmul(ps_o[:], h_kxm[:, k:k + 1],
                                     wps[v][:, k * 512:(k + 1) * 512],
                                     start=(k == 0), stop=(k == 7))
                lst = apool.tile([1, 512], f32, name="lst", tag="lst", bufs=2)
                nc.vector.tensor_copy(lst[:], ps_o[:])
                nc.scalar.dma_start(d_logits[:, v * 512:(v + 1) * 512], lst[:])

    nc.compile()
    return nc


def _prep_inputs(input_idx, h0, c0, encoder_outputs, emb, attn_W, attn_b,
                 comb_W, comb_b, w_ih, w_hh, b_ih, b_hh, out_W, out_b):
    f = np.float32
    idx = int(np.asarray(input_idx).reshape(-1)[0])
    embedded = np.asarray(emb, f)[idx]                    # [H]
    h0v = np.asarray(h0, f).reshape(H)
    c0v = np.asarray(c0, f).reshape(H)

    ain = _pack_kxm(np.concatenate([embedded, h0v]))      # [128, 16]
    attn_w = _pack_w(np.asarray(attn_W, f).T)             # [128, 16*512]
    enc = _pack_w(np.asarray(encoder_outputs, f))         # [128, 4*1024]
    comb_w = _pack_w(np.asarray(comb_W, f).T)             # [128, 16*1024]
    attn_bv = np.asarray(attn_b, f).reshape(1, 512)
    comb_bv = np.asarray(comb_b, f).reshape(1, 1024)

    w_cat = np.concatenate([np.asarray(w_ih, f).T,
                            np.asarray(w_hh, f).T], axis=0)   # [2048, 4096]
    b_cat = (np.asarray(b_ih, f) + np.asarray(b_hh, f))       # [4096]

    out_WT = np.asarray(out_W, f).T                       # [1024, VOCAB]
    out_WT_pad = np.zeros((H, NCORES * VS), f)
    out_WT_pad[:, :VOCAB] = out_WT
    out_b_pad = np.zeros(NCORES * VS, f)
    out_b_pad[:VOCAB] = np.asarray(out_b, f)

    cst_all = np.zeros((128, 32), f)
    cst_all[:, 0:16] = ain.astype(f)
    cst_all[0, 16] = 1.0
    cst_all[0:8, 17:25] = np.eye(8, dtype=f)
    cst_all = cst_all.astype(NPW)

    in_maps = []
    for r in range(NCORES):
        cols = np.concatenate(
            [np.arange(g * H + r * 128, g * H + (r + 1) * 128)
             for g in range(4)])
        lstm_w = _pack_w(np.ascontiguousarray(w_cat[:, cols]))  # [128,16*512]
        lstm_b = b_cat[cols].reshape(1, 512)
        wv = out_WT_pad[:, r * VS:(r + 1) * VS]           # [1024, VS]
        wout = np.ascontiguousarray(
            wv.reshape(8, 128, NVT, 512).transpose(2, 1, 0, 3)
        ).reshape(NVT, 128, 8 * 512).astype(NPO)
        bias_all = np.concatenate([
            attn_bv.reshape(-1), comb_bv.reshape(-1), lstm_b.reshape(-1),
            c0v[r * 128:(r + 1) * 128]]).reshape(1, 2176).astype(f)
        in_maps.append({
            "cst_all": cst_all, "bias_all": bias_all,
            "attn_w": attn_w, "enc": enc, "comb_w": comb_w,
            "lstm_w": lstm_w, "wout": wout,
        })
    return in_maps, out_b_pad


def run_on_device(in_maps, trace=False):
    if "nc" not in _cache:
        _cache["nc"] = _build()
    nc = _cache["nc"]
    return bass_utils.run_bass_kernel_spmd(
        nc, in_maps, core_ids=list(range(NCORES)), trace=trace)


def kernel(**inputs):
    in_maps, out_b_pad = _prep_inputs(**inputs)
    res = run_on_device(in_maps).results

    logits = np.concatenate(
        [res[r]["logits_o"].reshape(VS) for r in range(NCORES)])[:VOCAB]
    logits = logits + out_b_pad[:VOCAB]
    m = float(logits.max())
    lse = m + float(np.log(np.exp(logits - m, dtype=np.float64).sum()))
    out = (logits - np.float32(lse)).astype(np.float32).reshape(1, VOCAB)

    h = np.concatenate([res[r]["h_o"].reshape(128) for r in range(NCORES)])
    c = np.concatenate([res[r]["c_o"].reshape(128) for r in range(NCORES)])
    attnw = res[0]["attnw_o"].reshape(1, SEQ).astype(np.float32)
    return (out,
            h.astype(np.float32).reshape(1, 1, H),
            c.astype(np.float32).reshape(1, 1, H),
            attnw)


# revision 19
# speedup vs baseline: 36278.8973x; 14.8685x over previous
"""AttnDecoderRNN single-step on 8 trn2 NeuronCores.

Strategy (tensor parallel over vocab, per sharding hint):
- Embedding lookup on host (only the one needed row of the 206MB table is
  ever read; shipping the table to the device would be pure waste).
- Attention + combine replicated on every core (small weights).
- LSTM cell sharded over the hidden dim: core r computes gate slices
  i/f/g/o[r*128:(r+1)*128] -> its h/c slice.
- AllGather of the 128-elem h slices -> full h on every core.
- Output projection sharded over vocab: core r computes logits for its
  6656 (padded) vocab columns. log_softmax normalization is finished on
  host (combines 8 shard stats; O(vocab) host work).

All matvecs run the activation vector as the PE stationary operand and
stream the weight matrix as the moving operand, 4-wide across PE column
groups (tile_position) with a DVE partial-row sum. Weights are host-packed
into the exact SBUF layout and streamed in [128, 4096]-element chunks
through one tile pool so weight DMA pipelines with PE; the out-projection
panels (fp8 e4m3 -- the logits have ~100x error headroom vs the h/c/attnw
outputs, which stay bf16) are issued 5-up-front + 8-deferred-on-h so the
AllGather's tiny DMA is not queued behind the bulk stream. Vector
transposes ([1,n*128] -> [128,n]) are PE K=1 matmuls against a ones
scalar (no DRAM bounce). Small/critical DMAs ride the ACT-HWDGE and
Pool-SWDGE queues, never the SP bulk-stream queue.
"""

import os
import numpy as np
import ml_dtypes

import concourse.bass as bass
import concourse.bacc as bacc
import concourse.mybir as mybir
import concourse.tile as tile
from concourse import bass_utils

NCORES = 8
H = 1024
SEQ = 512
VOCAB = 50257
VS = 6656               # padded per-core vocab shard; 8*VS = 53248
NVT = VS // 512         # 13 weight panels of [128, 8*512] per core
CHUNK = 4096            # streaming chunk free-dim (8KB/partition bf16)
KD = os.environ.get("KDTYPE", "bf16")
NPW = ml_dtypes.bfloat16 if KD == "bf16" else np.float32
WBUFS = int(os.environ.get("KWBUFS", "9"))
KOUT8 = os.environ.get("KOUT8", "1") == "1"   # fp8 e4m3 out-projection stream
KPACK = os.environ.get("KPACK", "1") == "1"   # 4-wide PE col-group packing
NPO = ml_dtypes.float8_e4m3 if KOUT8 else NPW

_cache = {}


def _pack_kxm(vec):
    """[C*128] vector -> [128, C] chunk-per-column layout (PE stationary)."""
    c = vec.shape[0] // 128
    return np.ascontiguousarray(vec.reshape(c, 128).T).astype(NPW)


def _pack_w(wT):
    """[K, N] weight (K = contraction, mult of 128) -> [128, (K/128)*N];
    column block k holds rows k*128:(k+1)*128."""
    k = wT.shape[0] // 128
    n = wT.shape[1]
    return np.ascontiguousarray(
        wT.reshape(k, 128, n).transpose(1, 0, 2).reshape(128, k * n)
    ).astype(NPW)


def _build(ablate=()):
    wdt = mybir.dt.bfloat16 if KD == "bf16" else mybir.dt.float32
    odt = mybir.dt.float8e4 if KOUT8 else wdt
    f32 = mybir.dt.float32
    AF = mybir.ActivationFunctionType
    OP = mybir.AluOpType

    nc = bacc.Bacc("TRN2", target_bir_lowering=False, debug=False,
                   num_devices=NCORES)

    def din(name, shape, dt=None):
        return nc.dram_tensor(name, shape, dt or wdt, kind="ExternalInput").ap()

    def dout(name, shape):
        return nc.dram_tensor(name, shape, f32, kind="ExternalOutput").ap()

    # cst_all: cols 0:16 = ain kxm, col 16 = ones, cols 17:25 rows 0:8 = eye8
    d_cst = din("cst_all", [128, 32])
    # bias_all: 0:512 attn_b | 512:1536 comb_b | 1536:2048 lstm_b | 2048:2176 c0
    d_bias = din("bias_all", [1, 2176], f32)
    d_attnw = din("attn_w", [128, 16 * 512])      # 2 chunks
    d_enc = din("enc", [128, 4 * 1024])           # 1 chunk
    d_combw = din("comb_w", [128, 16 * 1024])     # 4 chunks
    d_lstmw = din("lstm_w", [128, 16 * 512])      # 2 chunks
    d_wout = din("wout", [NVT, 128, 8 * 512], odt)   # 13 panels

    d_logits = dout("logits_o", [1, VS])
    d_ho = dout("h_o", [1, 128])
    d_co = dout("c_o", [1, 128])
    d_awo = dout("attnw_o", [1, 512])

    with tile.TileContext(nc) as tc:
        with (
            tc.tile_pool(name="const", bufs=1) as cpool,
            tc.tile_pool(name="wpool", bufs=WBUFS) as wpool,
            tc.tile_pool(name="act", bufs=1) as apool,
            tc.tile_pool(name="ps", bufs=2, space="PSUM") as pspool,
            tc.tile_pool(name="pst", bufs=2, space="PSUM") as pstpool,
            tc.tile_pool(name="pso", bufs=2, space="PSUM") as psopool,
            tc.tile_pool(name="dram", bufs=1, space="DRAM") as dpool,
        ):
            # ---- two merged constant loads (unblock phase-1 chain fast)
            cst = cpool.tile([128, 32], wdt)
            nc.scalar.dma_start(cst[:], d_cst[:])
            bias = cpool.tile([1, 2176], f32)
            nc.scalar.dma_start(bias[:], d_bias[:])
            ain = cst[:, 0:16]
            one = cst[0:1, 16:17]
            eye8 = cst[0:8, 17:25]
            ab = bias[:, 0:512]
            cb = bias[:, 512:1536]
            lb = bias[:, 1536:2048]
            c0s = bias[:, 2048:2176]

            def wchunk(src_ap, name, dt=None, tag="ws", bufs=9):
                t = wpool.tile([128, CHUNK], dt or wdt, name=name, tag=tag,
                               bufs=bufs)
                nc.sync.dma_start(t[:], src_ap)
                return t

            # ---- streamed weight chunks, program order = DMA priority
            aw0 = wchunk(d_attnw[:, 0:CHUNK], "aw0")
            aw1 = wchunk(d_attnw[:, CHUNK:2 * CHUNK], "aw1")
            enc = wchunk(d_enc[:], "enc")
            cwc = [wchunk(d_combw[:, c * CHUNK:(c + 1) * CHUNK], f"cw{c}")
                   for c in range(4)]
            lw0 = wchunk(d_lstmw[:, 0:CHUNK], "lw0")
            lw1 = wchunk(d_lstmw[:, CHUNK:2 * CHUNK], "lw1")
            NPRE = int(os.environ.get("KNPRE", "5"))
            wps = [wchunk(d_wout[v], f"wp{v}", odt, "wo", NVT)
                   for v in range(NPRE)]

            def vec_to_kxm(src_f32_ap, n, name):
                """[1, n*128] f32 -> [128, n] wdt via K=1 PE matmuls."""
                vc = apool.tile([1, n * 128], wdt, name=name + "_c")
                nc.vector.tensor_copy(vc[:], src_f32_ap)
                pst = pstpool.tile([128, 8], f32, name=name + "_p", tag="pst")
                for c in range(n):
                    nc.tensor.matmul(pst[:, c:c + 1],
                                     vc[:, c * 128:(c + 1) * 128], one,
                                     start=True, stop=True)
                kxm = apool.tile([128, n], wdt, name=name + "_k")
                nc.vector.tensor_copy(kxm[:], pst[:, 0:n])
                return kxm

            def packed_matvec(ps, nsl, lhs_of, rhs_of, nk, name):
                """sum_k lhs[k].T @ rhs[k] -> [1, 512] into sbuf tile.
                ps: [128, >=nsl*512] psum tile; 4 col groups accumulate
                in parallel rounds; partial rows 0/32/64/96 summed on DVE."""
                outs = []
                for n in range(nsl):
                    sl = slice(n * 512, (n + 1) * 512)
                    if KPACK and nk >= 8:
                        nr = (nk + 3) // 4
                        for r in range(nr):
                            for j in range(4):
                                k = r * 4 + j
                                if k >= nk:
                                    break
                                nc.tensor.matmul(
                                    ps[32 * j:32 * j + 1, sl], lhs_of(k),
                                    rhs_of(k, n), start=(r == 0),
                                    stop=(r >= nr - 1),
                                    tile_position=(0, 32 * j))
                        o = apool.tile([1, 512], f32, name=f"{name}o_{n}",
                                       tag=f"{name}o")
                        nc.vector.tensor_copy(o[:], ps[0:1, sl])
                        for j in range(1, 4):
                            nc.vector.tensor_add(o[:], o[:],
                                                 ps[32 * j:32 * j + 1, sl])
                        outs.append(o)
                    else:
                        for k in range(nk):
                            nc.tensor.matmul(ps[0:1, sl], lhs_of(k),
                                             rhs_of(k, n), start=(k == 0),
                                             stop=(k == nk - 1))
                        o = apool.tile([1, 512], f32, name=f"{name}o_{n}",
                                       tag=f"{name}o")
                        nc.vector.tensor_copy(o[:], ps[0:1, sl])
                        outs.append(o)
                return outs

            # ---------- stage 1: attention scores + softmax [1,512]
            ps_s = pspool.tile([128, 1024], f32, name="ps_s", tag="ps")
            (s_o,) = packed_matvec(
                ps_s, 1,
                lambda k: ain[:, k:k + 1],
                lambda k, n: (aw0 if k < 8 else aw1)[:, (k % 8) * 512:
                                                    (k % 8 + 1) * 512],
                16, "sc")
            scores = apool.tile([1, 512], f32)
            nc.vector.tensor_add(scores[:], s_o[:], ab)
            mx = apool.tile([1, 1], f32)
            nc.vector.reduce_max(mx[:], scores[:], axis=mybir.AxisListType.X)
            nmx = apool.tile([1, 1], f32)
            nc.vector.tensor_scalar_mul(nmx[:], mx[:], -1.0)
            expv = apool.tile([1, 512], f32)
            ssum = apool.tile([1, 1], f32)
            nc.scalar.activation(expv[:], scores[:], AF.Exp, bias=nmx[:],
                                 scale=1.0, accum_out=ssum[:])
            rs = apool.tile([1, 1], f32)
            nc.vector.reciprocal(rs[:], ssum[:])
            awf = apool.tile([1, 512], f32)
            nc.vector.tensor_scalar_mul(awf[:], expv[:], rs[:])
            aw_kxm = vec_to_kxm(awf[:], 4, "awr")

            # ---------- stage 2: attn_applied [1,1024]
            ps_a = pspool.tile([128, 1024], f32, name="ps_a", tag="ps")
            a_os = packed_matvec(
                ps_a, 2,
                lambda k: aw_kxm[:, k:k + 1],
                lambda k, n: enc[:, k * 1024 + n * 512:
                                 k * 1024 + (n + 1) * 512],
                4, "aa")
            aav = apool.tile([1, 1024], f32)
            nc.vector.tensor_copy(aav[:, 0:512], a_os[0][:])
            nc.vector.tensor_copy(aav[:, 512:1024], a_os[1][:])
            aa_kxm = vec_to_kxm(aav[:], 8, "aar")

            # ---------- stage 3: combine + relu -> x [1,1024]
            ps_x = pspool.tile([128, 1024], f32, name="ps_x", tag="ps")

            def cw_rhs(k, n):
                c, kk = divmod(k * 1024 + n * 512, CHUNK)
                return cwc[c][:, kk:kk + 512]

            x_os = packed_matvec(
                ps_x, 2,
                lambda k: ain[:, k:k + 1] if k < 8 else aa_kxm[:, k - 8:k - 7],
                cw_rhs, 16, "xx")
            xb = apool.tile([1, 1024], f32)
            nc.vector.tensor_add(xb[:, 0:512], x_os[0][:], cb[:, 0:512])
            nc.vector.tensor_add(xb[:, 512:1024], x_os[1][:], cb[:, 512:1024])
            xr = apool.tile([1, 1024], f32)
            nc.vector.tensor_scalar_max(xr[:], xb[:], 0.0)
            x_kxm = vec_to_kxm(xr[:], 8, "xr")

            # ---------- stage 4: LSTM gate slices [1,512] = [i|f|g|o]x128
            ps_g = pspool.tile([128, 1024], f32, name="ps_g", tag="ps")
            (g_o,) = packed_matvec(
                ps_g, 1,
                lambda k: x_kxm[:, k:k + 1] if k < 8 else ain[:, k:k + 1],
                lambda k, n: (lw0 if k < 8 else lw1)[:, (k % 8) * 512:
                                                    (k % 8 + 1) * 512],
                16, "gg")
            gt = apool.tile([1, 512], f32)
            nc.vector.tensor_add(gt[:], g_o[:], lb)
            acts = apool.tile([1, 512], f32)
            nc.scalar.activation(acts[:, 0:256], gt[:, 0:256], AF.Sigmoid)
            nc.scalar.activation(acts[:, 256:384], gt[:, 256:384], AF.Tanh)
            nc.scalar.activation(acts[:, 384:512], gt[:, 384:512], AF.Sigmoid)
            fc = apool.tile([1, 128], f32)
            nc.vector.tensor_mul(fc[:], acts[:, 128:256], c0s)
            ig = apool.tile([1, 128], f32)
            nc.vector.tensor_mul(ig[:], acts[:, 0:128], acts[:, 256:384])
            cn = apool.tile([1, 128], f32)
            nc.vector.tensor_add(cn[:], fc[:], ig[:])
            tch = apool.tile([1, 128], f32)
            nc.scalar.activation(tch[:], cn[:], AF.Tanh)
            hn = apool.tile([1, 128], f32)
            hn_inst = nc.vector.tensor_mul(hn[:], acts[:, 384:512], tch[:])
            for v in range(NPRE, NVT):
                t = wpool.tile([128, CHUNK], odt, name=f"wp{v}", tag="wo",
                               bufs=NVT)
                dls = nc.sync.dma_start(t[:], d_wout[v])
                tile.add_dep_helper(getattr(dls, "ins", dls),
                                    getattr(hn_inst, "ins", hn_inst), sync=True)
                wps.append(t)

            # ---------- AllGather h slices -> full h, transpose to [128, 8]
            hc = apool.tile([1, 128], wdt)
            nc.vector.tensor_copy(hc[:], hn[:])
            ag_in = dpool.tile([1, 128], wdt)
            ag_out = dpool.tile([8, 128], wdt, addr_space="Shared")
            nc.gpsimd.dma_start(ag_in[:], hc[:])
            if "noag" not in ablate:
                nc.gpsimd.collective_compute(
                    "AllGather", OP.bypass,
                    replica_groups=[list(range(NCORES))],
                    ins=[ag_in[:]], outs=[ag_out[:]])
            hsb = apool.tile([8, 128], wdt)
            nc.gpsimd.dma_start(hsb[:], ag_out[:])
            ps_h = pstpool.tile([128, 8], f32, name="ps_h", tag="pst")
            nc.tensor.matmul(ps_h[:], hsb[:], eye8, start=True, stop=True)
            h_kxm = apool.tile([128, 8], odt)
            nc.vector.tensor_copy(h_kxm[:], ps_h[:])

            nc.gpsimd.dma_start(d_awo[:], awf[:])
            nc.gpsimd.dma_start(d_co[:], cn[:])
            nc.gpsimd.dma_start(d_ho[:], hn[:])

            # ---------- stage 5: vocab-sharded output projection
            # (out_b is added on host with the shard-combining log_softmax)
            for v in range(NVT):
                ps_o = psopool.tile([1, 512], f32, name="ps_o", tag="pso")
                for k in range(8):
                    nc.tensor.matmul(ps_o[:], h_kxm[:, k:k + 1],
                                     wps[v][:, k * 512:(k + 1) * 512],
                                     start=(k == 0), stop=(k == 7))
                lst = apool.tile([1, 512], f32, name="lst", tag="lst", bufs=2)
                nc.vector.tensor_copy(lst[:], ps_o[:])
                nc.scalar.dma_start(d_logits[:, v * 512:(v + 1) * 512], lst[:])

    nc.compile()
    return nc


def _prep_inputs(input_idx, h0, c0, encoder_outputs, emb, attn_W, attn_b,
                 comb_W, comb_b, w_ih, w_hh, b_ih, b_hh, out_W, out_b):
    f = np.float32
    idx = int(np.asarray(input_idx).reshape(-1)[0])
    embedded = np.asarray(emb, f)[idx]                    # [H]
    h0v = np.asarray(h0, f).reshape(H)
    c0v = np.asarray(c0, f).reshape(H)

    ain = _pack_kxm(np.concatenate([embedded, h0v]))      # [128, 16]
    attn_w = _pack_w(np.asarray(attn_W, f).T)             # [128, 16*512]
    enc = _pack_w(np.asarray(encoder_outputs, f))         # [128, 4*1024]
    comb_w = _pack_w(np.asarray(comb_W, f).T)             # [128, 16*1024]
    attn_bv = np.asarray(attn_b, f).reshape(1, 512)
    comb_bv = np.asarray(comb_b, f).reshape(1, 1024)

    w_cat = np.concatenate([np.asarray(w_ih, f).T,
                            np.asarray(w_hh, f).T], axis=0)   # [2048, 4096]
    b_cat = (np.asarray(b_ih, f) + np.asarray(b_hh, f))       # [4096]

    out_WT = np.asarray(out_W, f).T                       # [1024, VOCAB]
    out_WT_pad = np.zeros((H, NCORES * VS), f)
    out_WT_pad[:, :VOCAB] = out_WT
    out_b_pad = np.zeros(NCORES * VS, f)
    out_b_pad[:VOCAB] = np.asarray(out_b, f)

    cst_all = np.zeros((128, 32), f)
    cst_all[:, 0:16] = ain.astype(f)
    cst_all[0, 16] = 1.0
    cst_all[0:8, 17:25] = np.eye(8, dtype=f)
    cst_all = cst_all.astype(NPW)

    in_maps = []
    for r in range(NCORES):
        cols = np.concatenate(
            [np.arange(g * H + r * 128, g * H + (r + 1) * 128)
             for g in range(4)])
        lstm_w = _pack_w(np.ascontiguousarray(w_cat[:, cols]))  # [128,16*512]
        lstm_b = b_cat[cols].reshape(1, 512)
        wv = out_WT_pad[:, r * VS:(r + 1) * VS]           # [1024, VS]
        wout = np.ascontiguousarray(
            wv.reshape(8, 128, NVT, 512).transpose(2, 1, 0, 3)
        ).reshape(NVT, 128, 8 * 512).astype(NPO)
        bias_all = np.concatenate([
            attn_bv.reshape(-1), comb_bv.reshape(-1), lstm_b.reshape(-1),
            c0v[r * 128:(r + 1) * 128]]).reshape(1, 2176).astype(f)
        in_maps.append({
            "cst_all": cst_all, "bias_all": bias_all,
            "attn_w": attn_w, "enc": enc, "comb_w": comb_w,
            "lstm_w": lstm_w, "wout": wout,
        })
    return in_maps, out_b_pad


def run_on_device(in_maps, trace=False):
    if "nc" not in _cache:
        _cache["nc"] = _build()
    nc = _cache["nc"]
    last = None
    for attempt in range(3):
        try:
            return bass_utils.run_bass_kernel_spmd(
                nc, in_maps, core_ids=list(range(NCORES)), trace=trace)
        except Exception as e:  # transient NRT_EXEC_UNIT_UNRECOVERABLE wedges
            last = e
            import time
            time.sleep(2.0 * (attempt + 1))
    raise last


def kernel(**inputs):
    in_maps, out_b_pad = _prep_inputs(**inputs)
    res = run_on_device(in_maps).results

    logits = np.concatenate(
        [res[r]["logits_o"].reshape(VS) for r in range(NCORES)])[:VOCAB]
    logits = logits + out_b_pad[:VOCAB]
    m = float(logits.max())
    lse = m + float(np.log(np.exp(logits - m, dtype=np.float64).sum()))
    out = (logits - np.float32(lse)).astype(np.float32).reshape(1, VOCAB)

    h = np.concatenate([res[r]["h_o"].reshape(128) for r in range(NCORES)])
    c = np.concatenate([res[r]["c_o"].reshape(128) for r in range(NCORES)])
    attnw = res[0]["attnw_o"].reshape(1, SEQ).astype(np.float32)
    return (out,
            h.astype(np.float32).reshape(1, 1, H),
            c.astype(np.float32).reshape(1, 1, H),
            attnw)


# revision 20
# speedup vs baseline: 37186.0334x; 1.0250x over previous
"""AttnDecoderRNN single-step on 8 trn2 NeuronCores.

Strategy (tensor parallel over vocab, per sharding hint):
- Embedding lookup on host (only the one needed row of the 206MB table is
  ever read; shipping the table to the device would be pure waste).
- Attention + combine replicated on every core (small weights).
- LSTM cell sharded over the hidden dim: core r computes gate slices
  i/f/g/o[r*128:(r+1)*128] -> its h/c slice.
- AllGather of the 128-elem h slices -> full h on every core.
- Output projection sharded over vocab: core r computes logits for its
  6656 (padded) vocab columns. log_softmax normalization is finished on
  host (combines 8 shard stats; O(vocab) host work).

All matvecs run the activation vector as the PE stationary operand and
stream the weight matrix as the moving operand, 4-wide across PE column
groups (tile_position) with a DVE partial-row sum. Weights are host-packed
into the exact SBUF layout and streamed in [128, 4096]-element chunks
through one tile pool so weight DMA pipelines with PE; the out-projection
panels (fp8 e4m3 -- the logits have ~100x error headroom vs the h/c/attnw
outputs, which stay bf16) all prefetch during phase 1; the AllGather's tiny
DMAs ride the Pool-SWDGE rings, separate from the bulk stream. Vector
transposes ([1,n*128] -> [128,n]) are PE K=1 matmuls against a ones
scalar (no DRAM bounce). Small/critical DMAs ride the ACT-HWDGE and
Pool-SWDGE queues, never the SP bulk-stream queue.
"""

import os
import numpy as np
import ml_dtypes

import concourse.bass as bass
import concourse.bacc as bacc
import concourse.mybir as mybir
import concourse.tile as tile
from concourse import bass_utils

NCORES = 8
H = 1024
SEQ = 512
VOCAB = 50257
VS = 6656               # padded per-core vocab shard; 8*VS = 53248
NVT = VS // 512         # 13 weight panels of [128, 8*512] per core
CHUNK = 4096            # streaming chunk free-dim (8KB/partition bf16)
KD = os.environ.get("KDTYPE", "bf16")
NPW = ml_dtypes.bfloat16 if KD == "bf16" else np.float32
WBUFS = int(os.environ.get("KWBUFS", "9"))
KOUT8 = os.environ.get("KOUT8", "1") == "1"   # fp8 e4m3 out-projection stream
KPACK = os.environ.get("KPACK", "1") == "1"   # 4-wide PE col-group packing
NPO = ml_dtypes.float8_e4m3 if KOUT8 else NPW

_cache = {}


def _pack_kxm(vec):
    """[C*128] vector -> [128, C] chunk-per-column layout (PE stationary)."""
    c = vec.shape[0] // 128
    return np.ascontiguousarray(vec.reshape(c, 128).T).astype(NPW)


def _pack_w(wT):
    """[K, N] weight (K = contraction, mult of 128) -> [128, (K/128)*N];
    column block k holds rows k*128:(k+1)*128."""
    k = wT.shape[0] // 128
    n = wT.shape[1]
    return np.ascontiguousarray(
        wT.reshape(k, 128, n).transpose(1, 0, 2).reshape(128, k * n)
    ).astype(NPW)


def _build(ablate=()):
    wdt = mybir.dt.bfloat16 if KD == "bf16" else mybir.dt.float32
    odt = mybir.dt.float8e4 if KOUT8 else wdt
    f32 = mybir.dt.float32
    AF = mybir.ActivationFunctionType
    OP = mybir.AluOpType

    nc = bacc.Bacc("TRN2", target_bir_lowering=False, debug=False,
                   num_devices=NCORES)

    def din(name, shape, dt=None):
        return nc.dram_tensor(name, shape, dt or wdt, kind="ExternalInput").ap()

    def dout(name, shape):
        return nc.dram_tensor(name, shape, f32, kind="ExternalOutput").ap()

    # cst_all: cols 0:16 = ain kxm, col 16 = ones, cols 17:25 rows 0:8 = eye8
    d_cst = din("cst_all", [128, 32])
    # bias_all: 0:512 attn_b | 512:1536 comb_b | 1536:2048 lstm_b | 2048:2176 c0
    d_bias = din("bias_all", [1, 2176], f32)
    d_attnw = din("attn_w", [128, 16 * 512])      # 2 chunks
    d_enc = din("enc", [128, 4 * 1024])           # 1 chunk
    d_combw = din("comb_w", [128, 16 * 1024])     # 4 chunks
    d_lstmw = din("lstm_w", [128, 16 * 512])      # 2 chunks
    d_wout = din("wout", [NVT, 128, 8 * 512], odt)   # 13 panels

    d_logits = dout("logits_o", [1, VS])
    d_ho = dout("h_o", [1, 128])
    d_co = dout("c_o", [1, 128])
    d_awo = dout("attnw_o", [1, 512])

    with tile.TileContext(nc) as tc:
        with (
            tc.tile_pool(name="const", bufs=1) as cpool,
            tc.tile_pool(name="wpool", bufs=WBUFS) as wpool,
            tc.tile_pool(name="act", bufs=1) as apool,
            tc.tile_pool(name="ps", bufs=2, space="PSUM") as pspool,
            tc.tile_pool(name="pst", bufs=2, space="PSUM") as pstpool,
            tc.tile_pool(name="pso", bufs=2, space="PSUM") as psopool,
            tc.tile_pool(name="dram", bufs=1, space="DRAM") as dpool,
        ):
            # ---- two merged constant loads (unblock phase-1 chain fast)
            cst = cpool.tile([128, 32], wdt)
            nc.scalar.dma_start(cst[:], d_cst[:])
            bias = cpool.tile([1, 2176], f32)
            nc.scalar.dma_start(bias[:], d_bias[:])
            ain = cst[:, 0:16]
            one = cst[0:1, 16:17]
            eye8 = cst[0:8, 17:25]
            ab = bias[:, 0:512]
            cb = bias[:, 512:1536]
            lb = bias[:, 1536:2048]
            c0s = bias[:, 2048:2176]

            def wchunk(src_ap, name, dt=None, tag="ws", bufs=9):
                t = wpool.tile([128, CHUNK], dt or wdt, name=name, tag=tag,
                               bufs=bufs)
                nc.sync.dma_start(t[:], src_ap)
                return t

            # ---- streamed weight chunks, program order = DMA priority
            aw0 = wchunk(d_attnw[:, 0:CHUNK], "aw0")
            aw1 = wchunk(d_attnw[:, CHUNK:2 * CHUNK], "aw1")
            enc = wchunk(d_enc[:], "enc")
            cwc = [wchunk(d_combw[:, c * CHUNK:(c + 1) * CHUNK], f"cw{c}")
                   for c in range(4)]
            lw0 = wchunk(d_lstmw[:, 0:CHUNK], "lw0")
            lw1 = wchunk(d_lstmw[:, CHUNK:2 * CHUNK], "lw1")
            NPRE = int(os.environ.get("KNPRE", "13"))
            wps = [wchunk(d_wout[v], f"wp{v}", odt, "wo", NVT)
                   for v in range(NPRE)]

            def vec_to_kxm(src_f32_ap, n, name):
                """[1, n*128] f32 -> [128, n] wdt via K=1 PE matmuls."""
                vc = apool.tile([1, n * 128], wdt, name=name + "_c")
                nc.vector.tensor_copy(vc[:], src_f32_ap)
                pst = pstpool.tile([128, 8], f32, name=name + "_p", tag="pst")
                for c in range(n):
                    nc.tensor.matmul(pst[:, c:c + 1],
                                     vc[:, c * 128:(c + 1) * 128], one,
                                     start=True, stop=True)
                kxm = apool.tile([128, n], wdt, name=name + "_k")
                nc.vector.tensor_copy(kxm[:], pst[:, 0:n])
                return kxm

            def packed_matvec(ps, nsl, lhs_of, rhs_of, nk, name):
                """sum_k lhs[k].T @ rhs[k] -> [1, 512] into sbuf tile.
                ps: [128, >=nsl*512] psum tile; 4 col groups accumulate
                in parallel rounds; partial rows 0/32/64/96 summed on DVE."""
                outs = []
                for n in range(nsl):
                    sl = slice(n * 512, (n + 1) * 512)
                    if KPACK and nk >= 8:
                        nr = (nk + 3) // 4
                        for r in range(nr):
                            for j in range(4):
                                k = r * 4 + j
                                if k >= nk:
                                    break
                                nc.tensor.matmul(
                                    ps[32 * j:32 * j + 1, sl], lhs_of(k),
                                    rhs_of(k, n), start=(r == 0),
                                    stop=(r >= nr - 1),
                                    tile_position=(0, 32 * j))
                        o = apool.tile([1, 512], f32, name=f"{name}o_{n}",
                                       tag=f"{name}o")
                        nc.vector.tensor_copy(o[:], ps[0:1, sl])
                        for j in range(1, 4):
                            nc.vector.tensor_add(o[:], o[:],
                                                 ps[32 * j:32 * j + 1, sl])
                        outs.append(o)
                    else:
                        for k in range(nk):
                            nc.tensor.matmul(ps[0:1, sl], lhs_of(k),
                                             rhs_of(k, n), start=(k == 0),
                                             stop=(k == nk - 1))
                        o = apool.tile([1, 512], f32, name=f"{name}o_{n}",
                                       tag=f"{name}o")
                        nc.vector.tensor_copy(o[:], ps[0:1, sl])
                        outs.append(o)
                return outs

            # ---------- stage 1: attention scores + softmax [1,512]
            ps_s = pspool.tile([128, 1024], f32, name="ps_s", tag="ps")
            (s_o,) = packed_matvec(
                ps_s, 1,
                lambda k: ain[:, k:k + 1],
                lambda k, n: (aw0 if k < 8 else aw1)[:, (k % 8) * 512:
                                                    (k % 8 + 1) * 512],
                16, "sc")
            scores = apool.tile([1, 512], f32)
            nc.vector.tensor_add(scores[:], s_o[:], ab)
            mx = apool.tile([1, 1], f32)
            nc.vector.reduce_max(mx[:], scores[:], axis=mybir.AxisListType.X)
            nmx = apool.tile([1, 1], f32)
            nc.vector.tensor_scalar_mul(nmx[:], mx[:], -1.0)
            expv = apool.tile([1, 512], f32)
            ssum = apool.tile([1, 1], f32)
            nc.scalar.activation(expv[:], scores[:], AF.Exp, bias=nmx[:],
                                 scale=1.0, accum_out=ssum[:])
            rs = apool.tile([1, 1], f32)
            nc.vector.reciprocal(rs[:], ssum[:])
            awf = apool.tile([1, 512], f32)
            nc.vector.tensor_scalar_mul(awf[:], expv[:], rs[:])
            aw_kxm = vec_to_kxm(awf[:], 4, "awr")

            # ---------- stage 2: attn_applied [1,1024]
            ps_a = pspool.tile([128, 1024], f32, name="ps_a", tag="ps")
            a_os = packed_matvec(
                ps_a, 2,
                lambda k: aw_kxm[:, k:k + 1],
                lambda k, n: enc[:, k * 1024 + n * 512:
                                 k * 1024 + (n + 1) * 512],
                4, "aa")
            aav = apool.tile([1, 1024], f32)
            nc.vector.tensor_copy(aav[:, 0:512], a_os[0][:])
            nc.vector.tensor_copy(aav[:, 512:1024], a_os[1][:])
            aa_kxm = vec_to_kxm(aav[:], 8, "aar")

            # ---------- stage 3: combine + relu -> x [1,1024]
            ps_x = pspool.tile([128, 1024], f32, name="ps_x", tag="ps")

            def cw_rhs(k, n):
                c, kk = divmod(k * 1024 + n * 512, CHUNK)
                return cwc[c][:, kk:kk + 512]

            x_os = packed_matvec(
                ps_x, 2,
                lambda k: ain[:, k:k + 1] if k < 8 else aa_kxm[:, k - 8:k - 7],
                cw_rhs, 16, "xx")
            xb = apool.tile([1, 1024], f32)
            nc.vector.tensor_add(xb[:, 0:512], x_os[0][:], cb[:, 0:512])
            nc.vector.tensor_add(xb[:, 512:1024], x_os[1][:], cb[:, 512:1024])
            xr = apool.tile([1, 1024], f32)
            nc.vector.tensor_scalar_max(xr[:], xb[:], 0.0)
            x_kxm = vec_to_kxm(xr[:], 8, "xr")

            # ---------- stage 4: LSTM gate slices [1,512] = [i|f|g|o]x128
            ps_g = pspool.tile([128, 1024], f32, name="ps_g", tag="ps")
            (g_o,) = packed_matvec(
                ps_g, 1,
                lambda k: x_kxm[:, k:k + 1] if k < 8 else ain[:, k:k + 1],
                lambda k, n: (lw0 if k < 8 else lw1)[:, (k % 8) * 512:
                                                    (k % 8 + 1) * 512],
                16, "gg")
            gt = apool.tile([1, 512], f32)
            nc.vector.tensor_add(gt[:], g_o[:], lb)
            acts = apool.tile([1, 512], f32)
            nc.scalar.activation(acts[:, 0:256], gt[:, 0:256], AF.Sigmoid)
            nc.scalar.activation(acts[:, 256:384], gt[:, 256:384], AF.Tanh)
            nc.scalar.activation(acts[:, 384:512], gt[:, 384:512], AF.Sigmoid)
            fc = apool.tile([1, 128], f32)
            nc.vector.tensor_mul(fc[:], acts[:, 128:256], c0s)
            ig = apool.tile([1, 128], f32)
            nc.vector.tensor_mul(ig[:], acts[:, 0:128], acts[:, 256:384])
            cn = apool.tile([1, 128], f32)
            nc.vector.tensor_add(cn[:], fc[:], ig[:])
            tch = apool.tile([1, 128], f32)
            nc.scalar.activation(tch[:], cn[:], AF.Tanh)
            hn = apool.tile([1, 128], f32)
            hn_inst = nc.vector.tensor_mul(hn[:], acts[:, 384:512], tch[:])
            for v in range(NPRE, NVT):
                t = wpool.tile([128, CHUNK], odt, name=f"wp{v}", tag="wo",
                               bufs=NVT)
                dls = nc.sync.dma_start(t[:], d_wout[v])
                tile.add_dep_helper(getattr(dls, "ins", dls),
                                    getattr(hn_inst, "ins", hn_inst), sync=True)
                wps.append(t)

            # ---------- AllGather h slices -> full h, transpose to [128, 8]
            hc = apool.tile([1, 128], wdt)
            nc.vector.tensor_copy(hc[:], hn[:])
            ag_in = dpool.tile([1, 128], wdt)
            ag_out = dpool.tile([8, 128], wdt, addr_space="Shared")
            nc.gpsimd.dma_start(ag_in[:], hc[:])
            if "noag" not in ablate:
                nc.gpsimd.collective_compute(
                    "AllGather", OP.bypass,
                    replica_groups=[list(range(NCORES))],
                    ins=[ag_in[:]], outs=[ag_out[:]])
            hsb = apool.tile([8, 128], wdt)
            nc.gpsimd.dma_start(hsb[:], ag_out[:])
            ps_h = pstpool.tile([128, 8], f32, name="ps_h", tag="pst")
            nc.tensor.matmul(ps_h[:], hsb[:], eye8, start=True, stop=True)
            h_kxm = apool.tile([128, 8], odt)
            nc.vector.tensor_copy(h_kxm[:], ps_h[:])

            nc.gpsimd.dma_start(d_awo[:], awf[:])
            nc.gpsimd.dma_start(d_co[:], cn[:])
            nc.gpsimd.dma_start(d_ho[:], hn[:])

            # ---------- stage 5: vocab-sharded output projection
            # (out_b is added on host with the shard-combining log_softmax)
            for v in range(NVT):
                ps_o = psopool.tile([1, 512], f32, name="ps_o", tag="pso")
                for k in range(8):
                    nc.tensor.matmul(ps_o[:], h_kxm[:, k:k + 1],
                                     wps[v][:, k * 512:(k + 1) * 512],
                                     start=(k == 0), stop=(k == 7))
                lst = apool.tile([1, 512], f32, name="lst", tag="lst", bufs=2)
                nc.vector.tensor_copy(lst[:], ps_o[:])
                nc.scalar.dma_start(d_logits[:, v * 512:(v + 1) * 512], lst[:])

    nc.compile()
    return nc


def _prep_inputs(input_idx, h0, c0, encoder_outputs, emb, attn_W, attn_b,
                 comb_W, comb_b, w_ih, w_hh, b_ih, b_hh, out_W, out_b):
    f = np.float32
    idx = int(np.asarray(input_idx).reshape(-1)[0])
    embedded = np.asarray(emb, f)[idx]                    # [H]
    h0v = np.asarray(h0, f).reshape(H)
    c0v = np.asarray(c0, f).reshape(H)

    ain = _pack_kxm(np.concatenate([embedded, h0v]))      # [128, 16]
    attn_w = _pack_w(np.asarray(attn_W, f).T)             # [128, 16*512]
    enc = _pack_w(np.asarray(encoder_outputs, f))         # [128, 4*1024]
    comb_w = _pack_w(np.asarray(comb_W, f).T)             # [128, 16*1024]
    attn_bv = np.asarray(attn_b, f).reshape(1, 512)
    comb_bv = np.asarray(comb_b, f).reshape(1, 1024)

    w_cat = np.concatenate([np.asarray(w_ih, f).T,
                            np.asarray(w_hh, f).T], axis=0)   # [2048, 4096]
    b_cat = (np.asarray(b_ih, f) + np.asarray(b_hh, f))       # [4096]

    out_WT = np.asarray(out_W, f).T                       # [1024, VOCAB]
    out_WT_pad = np.zeros((H, NCORES * VS), f)
    out_WT_pad[:, :VOCAB] = out_WT
    out_b_pad = np.zeros(NCORES * VS, f)
    out_b_pad[:VOCAB] = np.asarray(out_b, f)

    cst_all = np.zeros((128, 32), f)
    cst_all[:, 0:16] = ain.astype(f)
    cst_all[0, 16] = 1.0
    cst_all[0:8, 17:25] = np.eye(8, dtype=f)
    cst_all = cst_all.astype(NPW)

    in_maps = []
    for r in range(NCORES):
        cols = np.concatenate(
            [np.arange(g * H + r * 128, g * H + (r + 1) * 128)
             for g in range(4)])
        lstm_w = _pack_w(np.ascontiguousarray(w_cat[:, cols]))  # [128,16*512]
        lstm_b = b_cat[cols].reshape(1, 512)
        wv = out_WT_pad[:, r * VS:(r + 1) * VS]           # [1024, VS]
        wout = np.ascontiguousarray(
            wv.reshape(8, 128, NVT, 512).transpose(2, 1, 0, 3)
        ).reshape(NVT, 128, 8 * 512).astype(NPO)
        bias_all = np.concatenate([
            attn_bv.reshape(-1), comb_bv.reshape(-1), lstm_b.reshape(-1),
            c0v[r * 128:(r + 1) * 128]]).reshape(1, 2176).astype(f)
        in_maps.append({
            "cst_all": cst_all, "bias_all": bias_all,
            "attn_w": attn_w, "enc": enc, "comb_w": comb_w,
            "lstm_w": lstm_w, "wout": wout,
        })
    return in_maps, out_b_pad


def run_on_device(in_maps, trace=False):
    if "nc" not in _cache:
        _cache["nc"] = _build()
    nc = _cache["nc"]
    last = None
    for attempt in range(3):
        try:
            return bass_utils.run_bass_kernel_spmd(
                nc, in_maps, core_ids=list(range(NCORES)), trace=trace)
        except Exception as e:  # transient NRT_EXEC_UNIT_UNRECOVERABLE wedges
            last = e
            import time
            time.sleep(2.0 * (attempt + 1))
    raise last


def kernel(**inputs):
    in_maps, out_b_pad = _prep_inputs(**inputs)
    res = run_on_device(in_maps).results

    logits = np.concatenate(
        [res[r]["logits_o"].reshape(VS) for r in range(NCORES)])[:VOCAB]
    logits = logits + out_b_pad[:VOCAB]
    m = float(logits.max())
    lse = m + float(np.log(np.exp(logits - m, dtype=np.float64).sum()))
    out = (logits - np.float32(lse)).astype(np.float32).reshape(1, VOCAB)

    h = np.concatenate([res[r]["h_o"].reshape(128) for r in range(NCORES)])
    c = np.concatenate([res[r]["c_o"].reshape(128) for r in range(NCORES)])
    attnw = res[0]["attnw_o"].reshape(1, SEQ).astype(np.float32)
    return (out,
            h.astype(np.float32).reshape(1, 1, H),
            c.astype(np.float32).reshape(1, 1, H),
            attnw)
